# revision 1
# baseline (speedup 1.0000x reference)
"""Trainium2 Bass kernel for nn_CustomAttentionLayer (sparse_attention).

Strategy (8 NeuronCores, SPMD single launch):
 - Shard the K=1024 query-cluster axis: core m owns rows [128m, 128m+128).
 - Phase 1 (DMA-bound, ~67MB/core): stream the per-core column slices of
   q_assignments / k_assignments (fp32 in HBM, cast to fp16 on the fly)
   through the PE array against the N-side feature matrix
   X = [one_hot(iso) | ones | coords] to get the (16,128) reductions
   (d_k_raw.T, sum, centroid numerators) for both q and k sides.
 - The k-side (16,128) slab is AllGather'd across the 8 cores (61KB) so every
   core has the full k-side (16,1024).
 - Phase 2: R = G*H computed with a (4i x 32c)-partition packing: the G/H MLP
   hidden activations are built with per-partition scalar bias/scale tricks,
   relu'd on ACT/DVE/GPSIMD, and contracted over the hidden channel axis on
   the PE array via constant sign-pattern matmuls that accumulate straight
   into (128,1024) G_inner / H_inner psum tiles. Attention + FFN + layernorms
   finish on-chip; each core writes its (128,256) slab of the output.
"""
import numpy as np

import concourse.bass as bass
import concourse.mybir as mybir
import concourse.tile as tile
from concourse.bass_utils import run_bass_kernel_spmd

F32 = mybir.dt.float32
F16 = mybir.dt.float16
AF = mybir.ActivationFunctionType
OP = mybir.AluOpType

NCORES = 8
K, D, N, NISO = 1024, 256, 65536, 12
KSL = K // NCORES           # 128 rows per core
NCH = N // 128              # 512 contraction chunks
GRP = 32                    # chunks per DMA group
NGRP = NCH // GRP           # 16
XC = 16                     # X feature cols: [oh(12) | ones | cx | cy]  (+0 pad)

_cache = {}


# ---------------------------------------------------------------------------
# walrus in this container rejects >1 sync wait per instruction; split extras
# onto single-wait NOPs on the same engine right before the instruction.
def _split_multiwaits(nc):
    ctr = 0
    for f in nc.m.functions:
        for bb in f.blocks:
            for inst in list(bb.instructions):
                si = inst.sync_info
                if si is None:
                    continue
                waits = list(si.on_wait)
                if len(waits) <= 1:
                    continue
                si.on_wait = [waits[-1]]
                pos = None
                for j, cur in enumerate(bb.instructions):
                    if cur.name == inst.name:
                        pos = j
                        break
                assert pos is not None
                for k2, w in enumerate(waits[:-1]):
                    nop = mybir.InstNoOp(
                        name=f"wsplit-{ctr}",
                        sync_info=mybir.SyncInfo(on_wait=[w], on_update=[]),
                        engine=inst.engine,
                        bass_nofuse=True,
                    )
                    ctr += 1
                    nc.register_instruction(nop)
                    bb.instructions.insert(pos + k2, nop)
    return ctr


def build_program():
    nc = bass.Bass()

    # ---------------- DRAM I/O ----------------
    def din(name, shape, dt=F32):
        return nc.dram_tensor(name, list(shape), dt, kind="ExternalInput")

    qa_t = din("qa_t", (128, NCH, 128))          # fp32, per-core A_q slice (transposed-chunked)
    ka_t = din("ka_t", (128, NCH, 128))
    qx_t = din("qx_t", (128, NCH, XC), F16)      # N-side features, fp16
    kx_t = din("kx_t", (128, NCH, XC), F16)
    qT3 = din("qT3", (128, 2, 128), F16)         # query[sl].T chunked
    keyT3 = din("keyT3", (128, 2, 1024), F16)    # key.T chunked
    key3 = din("key3", (128, 8, 256), F16)       # key chunked
    q_sb_h = din("q_sbh", (128, 256))            # query[sl] fp32
    g1ab_h = din("g1ab", (13, 32))               # [G1A * |w2| ; g1_b * |w2|]
    g1b4_h = din("g1b4", (12, 128))              # G1B * |w2| tiled x4
    cc_g_h = din("cc_g", (128, 256), F16)        # sign-pattern for G contraction
    cc_h_h = din("cc_h", (128, 256), F16)
    bc32_h = din("bc32", (128, 8, 128), F16)      # row-broadcast patterns
    wq_h = din("wq_h", (1, 32))                  # h1_w[1]*|h2w|
    b1_h = din("b1_h", (1, 32))                  # h1_b*|h2w|
    wk4_h = din("wk4", (1, 128))                 # h1_w[2]*|h2w| tiled x4
    aH4_h = din("aH4", (128, 1))                 # h1_w[0]*|h2w| tiled x4 (col)
    g2b_h = din("g2b", (128, 1))
    h2b_h = din("h2b", (128, 1))
    ones_h = din("ones_r", (1, 128))
    sens_h = din("sens", (1, 2))
    sensr_h = din("sensr", (128, 2))
    i32_h = din("i128f", (128, 128))             # fp32 identity
    i16_h = din("i128h", (128, 128), F16)        # fp16 identity
    f1w_h = din("f1w", (128, 2, 8, 128), F16)
    f1b_h = din("f1b", (128, 8))
    f2w_h = din("f2w", (128, 8, 256), F16)
    f2b_h = din("f2br", (128, 256))
    l1g_h = din("l1g", (128, 256))
    l1b_h = din("l1b", (128, 256))
    l2g_h = din("l2g", (128, 256))
    l2b_h = din("l2b", (128, 256))
    eps_h = din("epsc", (128, 1))

    out_d = nc.dram_tensor("out", [128, 256], F32, kind="ExternalOutput")

    kside_d = nc.dram_tensor("kside", [16, 128], F32)
    kall_d = nc.dram_tensor("kall", [128, 128], F32, addr_space="Shared")

    with tile.TileContext(nc) as tc:
        with (
            tc.tile_pool(name="consts", bufs=1) as cp,
            tc.tile_pool(name="ph1", bufs=3) as p1,
            tc.tile_pool(name="sb", bufs=1) as sp,
            tc.tile_pool(name="sb2", bufs=2) as sp2,
            tc.tile_pool(name="pp", bufs=2, space="PSUM") as pp,
        ):
            # ---------------- const loads ----------------
            def cload(dram, dt=None, tag=None):
                t = cp.tile(list(dram.shape), dt or dram.dtype, tag=tag or dram.name)
                nc.sync.dma_start(out=t[:], in_=dram[:])
                return t

            qT3_s = cload(qT3)
            keyT3_s = cload(keyT3)
            key3_s = cload(key3)
            qsbh_s = cload(q_sb_h)
            g1ab_s = cload(g1ab_h)
            g1b4_s = cload(g1b4_h)
            ccg_s = cload(cc_g_h)
            cch_s = cload(cc_h_h)
            bc32_s = cload(bc32_h)
            wq_s = cload(wq_h)
            b1_s = cload(b1_h)
            wk4_s = cload(wk4_h)
            aH4_s = cload(aH4_h)
            g2b_s = cload(g2b_h)
            h2b_s = cload(h2b_h)
            ones_s = cload(ones_h)
            sens_s = cload(sens_h)
            sensr_s = cload(sensr_h)
            i32_s = cload(i32_h)
            i16_s = cload(i16_h)
            f1w_s = cload(f1w_h)
            f1b_s = cload(f1b_h)
            f2w_s = cload(f2w_h)
            f2b_s = cload(f2b_h)
            l1g_s = cload(l1g_h)
            l1b_s = cload(l1b_h)
            l2g_s = cload(l2g_h)
            l2b_s = cload(l2b_h)
            eps_s = cload(eps_h)

            # ---------------- phase 1: big reductions ----------------
            def big_reduce(a_dram, x_dram, ps, nm):
                for g in range(NGRP):
                    at = p1.tile([128, GRP, 128], F16, tag=f"a_{nm}")
                    nc.gpsimd.dma_start(out=at[:], in_=a_dram[:, g * GRP:(g + 1) * GRP, :])
                    xt = p1.tile([128, GRP, XC], F16, tag=f"x_{nm}")
                    nc.sync.dma_start(out=xt[:], in_=x_dram[:, g * GRP:(g + 1) * GRP, :])
                    for c in range(GRP):
                        nc.tensor.matmul(
                            ps[:], lhsT=xt[:, c, :], rhs=at[:, c, :],
                            start=(g == 0 and c == 0), stop=(g == NGRP - 1 and c == GRP - 1),
                        )

            # k first: its collective exchange overlaps the q reduction
            psk = pp.tile([16, 128], F32, tag="gh")
            big_reduce(ka_t, kx_t, psk, "k")

            # exchange k-side
            ksb = sp.tile([16, 128], F32, tag="ksb")
            nc.scalar.activation(ksb[:], psk[:], AF.Copy)
            nc.sync.dma_start(out=kside_d[:], in_=ksb[:])
            nc.gpsimd.collective_compute(
                "AllGather", OP.bypass,
                replica_groups=[list(range(NCORES))],
                ins=[kside_d[:]],
                outs=[kall_d[:]],
            )
            kview = kall_d.rearrange("(g c) k -> c g k", c=16)
            dkpT = sp.tile([12, 1024], F32, tag="dkpT")
            nc.sync.dma_start(out=dkpT[:].rearrange("c (g k) -> c g k", g=8),
                              in_=kview[0:12, :, :])
            ksum_r = sp.tile([1, 1024], F32, tag="ksum_r")
            nc.sync.dma_start(out=ksum_r[:].rearrange("c (g k) -> c g k", g=8),
                              in_=kview[12:13, :, :])
            kcx_r = sp.tile([1, 1024], F32, tag="kcx_r")
            nc.sync.dma_start(out=kcx_r[:].rearrange("c (g k) -> c g k", g=8),
                              in_=kview[13:14, :, :])
            kcy_r = sp.tile([1, 1024], F32, tag="kcy_r")
            nc.sync.dma_start(out=kcy_r[:].rearrange("c (g k) -> c g k", g=8),
                              in_=kview[14:15, :, :])

            # ---- k-side setup (overlaps q big_reduce) ----
            # rk = 1/(ksum+eps)  in place
            nc.vector.tensor_scalar_add(ksum_r[:], ksum_r[:], 1e-6)
            nc.vector.reciprocal(ksum_r[:], ksum_r[:])
            # centroids in place
            nc.vector.tensor_mul(kcx_r[:], kcx_r[:], ksum_r[:])
            nc.vector.tensor_mul(kcy_r[:], kcy_r[:], ksum_r[:])
            # nkps row
            s1 = sp.tile([1, 1024], F32, tag="s1")
            s2 = sp.tile([1, 1024], F32, tag="s2")
            nc.vector.tensor_scalar_sub(s1[:], kcx_r[:], sens_s[0:1, 0:1])
            nc.vector.tensor_scalar_sub(s2[:], kcy_r[:], sens_s[0:1, 1:2])
            nc.vector.tensor_mul(s1[:], s1[:], s1[:])
            nc.vector.tensor_mul(s2[:], s2[:], s2[:])
            nc.vector.tensor_add(s1[:], s1[:], s2[:])
            nkps_r = sp.tile([1, 1024], F32, tag="nkps_r")
            nc.scalar.activation(nkps_r[:], s1[:], AF.Sqrt)
            # dkp normalized: dkpT_n = dkpT * (ones x rk)
            rkb = pp.tile([12, 1024], F32, tag="wide")
            for b in range(2):
                nc.tensor.matmul(rkb[:, 512 * b:512 * (b + 1)], lhsT=ones_s[0:1, 0:12],
                                 rhs=ksum_r[0:1, 512 * b:512 * (b + 1)], start=True, stop=True)
            dkpn = sp.tile([12, 1024], F32, tag="dkpn")
            nc.vector.tensor_mul(dkpn[:], dkpT[:], rkb[:])
            # B4 = (G1B'|w2|).T @ dkp_n   -> (128, 1024)
            psB = pp.tile([128, 1024], F32, tag="wide")
            for b in range(2):
                nc.tensor.matmul(psB[:, 512 * b:512 * (b + 1)], lhsT=g1b4_s[:],
                                 rhs=dkpn[:, 512 * b:512 * (b + 1)], start=True, stop=True)
            B4 = sp.tile([128, 1024], F32, tag="B4")
            nc.scalar.activation(B4[:], psB[:], AF.Copy)
            # T24 = wk4 x nkps  -> sbuf (128,1024)
            psT2 = pp.tile([128, 1024], F32, tag="wide")
            for b in range(2):
                nc.tensor.matmul(psT2[:, 512 * b:512 * (b + 1)], lhsT=wk4_s[:],
                                 rhs=nkps_r[0:1, 512 * b:512 * (b + 1)], start=True, stop=True)
            T24 = sp.tile([128, 1024], F32, tag="T24")
            nc.scalar.activation(T24[:], psT2[:], AF.Copy)

            # q reduction second
            psq = pp.tile([16, 128], F32, tag="gh")
            big_reduce(qa_t, qx_t, psq, "q")

            # logits_raw = (q @ key.T)/sqrt(D)  (fp16 matmul, emitted late but
            # only depends on const tiles; scheduler fits it into phase 1)
            psl = pp.tile([128, 1024], F32, tag="wide")
            for c in range(2):
                for b in range(2):
                    nc.tensor.matmul(psl[:, 512 * b:512 * (b + 1)], lhsT=qT3_s[:, c, :],
                                     rhs=keyT3_s[:, c, 512 * b:512 * (b + 1)],
                                     start=(c == 0), stop=(c == 1))
            lraw = sp.tile([128, 1024], F32, tag="lraw")
            nc.scalar.activation(lraw[:], psl[:], AF.Copy, scale=1.0 / 16.0)

            # ---- q-side setup ----
            qsb = sp.tile([16, 128], F32, tag="qsb")
            nc.scalar.activation(qsb[:], psq[:], AF.Copy)
            pqT = pp.tile([128, 16], F32, tag="gh")
            nc.tensor.transpose(pqT[:], qsb[:], i32_s[0:16, 0:16])
            qT = sp.tile([128, 16], F32, tag="qT")
            nc.vector.tensor_copy(qT[:], pqT[:])
            rq_c = sp.tile([128, 1], F32, tag="rq_c")
            nc.vector.tensor_scalar_add(rq_c[:], qT[:, 12:13], 1e-6)
            nc.vector.reciprocal(rq_c[:], rq_c[:])
            qc2 = sp.tile([128, 2], F32, tag="qc2")
            nc.vector.tensor_scalar_mul(qc2[:], qT[:, 13:15], rq_c[:])
            nqc2 = sp.tile([128, 2], F32, tag="nqc2")
            nc.vector.tensor_scalar_mul(nqc2[:], qc2[:], -1.0)
            # n_ks col
            d2 = sp.tile([128, 2], F32, tag="d2")
            nc.vector.tensor_sub(d2[:], qc2[:], sensr_s[:])
            nc.vector.tensor_mul(d2[:], d2[:], d2[:])
            nks_c = sp.tile([128, 1], F32, tag="nks_c")
            nc.vector.tensor_reduce(nks_c[:], d2[:], mybir.AxisListType.X, OP.add)
            nc.scalar.activation(nks_c[:], nks_c[:], AF.Sqrt)
            # rows: rq_row, nks_row
            prow = pp.tile([1, 128], F32, tag="gh")
            nc.tensor.transpose(prow[:], rq_c[:], i32_s[:])
            rq_r = sp.tile([1, 128], F32, tag="rq_r")
            nc.vector.tensor_copy(rq_r[:], prow[:])
            prow2 = pp.tile([1, 128], F32, tag="gh")
            nc.tensor.transpose(prow2[:], nks_c[:], i32_s[:])
            nks_r = sp.tile([1, 128], F32, tag="nks_r")
            nc.vector.tensor_copy(nks_r[:], prow2[:])
            # normalized qsb rows 0:13 (row 12 = qsum*rq = 1 -> bias row)
            rqb = pp.tile([13, 128], F32, tag="gh")
            nc.tensor.matmul(rqb[:], lhsT=ones_s[0:1, 0:13], rhs=rq_r[:], start=True, stop=True)
            qsbn = sp.tile([13, 128], F32, tag="qsbn")
            nc.vector.tensor_mul(qsbn[:], qsb[0:13, :], rqb[:])
            # AT4 (128, 32): packed A'' bias
            psA = pp.tile([128, 32], F32, tag="gh")
            for ii in range(4):
                nc.tensor.matmul(psA[32 * ii:32 * (ii + 1), :], lhsT=g1ab_s[:],
                                 rhs=qsbn[:, ii::4], start=True, stop=True,
                                 tile_position=(0, 32 * ii))
            AT4 = sp.tile([128, 32], F32, tag="AT4")
            nc.vector.tensor_copy(AT4[:], psA[:])
            # T14 (128, 32): packed H bias
            psT1 = pp.tile([128, 32], F32, tag="gh")
            for ii in range(4):
                nc.tensor.matmul(psT1[32 * ii:32 * (ii + 1), :], lhsT=wq_s[:],
                                 rhs=nks_r[0:1, ii::4], start=True, stop=False,
                                 tile_position=(0, 32 * ii))
                nc.tensor.matmul(psT1[32 * ii:32 * (ii + 1), :], lhsT=b1_s[:],
                                 rhs=ones_s[0:1, ii::4], start=False, stop=True,
                                 tile_position=(0, 32 * ii))
            T14 = sp.tile([128, 32], F32, tag="T14")
            nc.vector.tensor_copy(T14[:], psT1[:])
            # n_kk (fp16): sqrt((kcx-qcx)^2 + (kcy-qcy)^2), i on partitions
            pKC = pp.tile([128, 1024], F32, tag="wide")
            for b in range(2):
                nc.tensor.matmul(pKC[:, 512 * b:512 * (b + 1)], lhsT=ones_s[:],
                                 rhs=kcx_r[0:1, 512 * b:512 * (b + 1)], start=True, stop=True)
            dx2 = sp.tile([128, 1024], F32, tag="dx2")
            nc.scalar.activation(dx2[:], pKC[:], AF.Square, bias=nqc2[:, 0:1])
            pKC2 = pp.tile([128, 1024], F32, tag="wide")
            for b in range(2):
                nc.tensor.matmul(pKC2[:, 512 * b:512 * (b + 1)], lhsT=ones_s[:],
                                 rhs=kcy_r[0:1, 512 * b:512 * (b + 1)], start=True, stop=True)
            dy2 = sp.tile([128, 1024], F32, tag="dy2")
            nc.scalar.activation(dy2[:], pKC2[:], AF.Square, bias=nqc2[:, 1:2])
            nc.vector.tensor_add(dx2[:], dx2[:], dy2[:])
            nkk16 = sp.tile([128, 1024], F16, tag="nkk16")
            nc.scalar.activation(nkk16[:], dx2[:], AF.Sqrt)

            # ---------------- phase 2 group loop ----------------
            gin = pp.tile([128, 1024], F32, tag="gh")
            hin = pp.tile([128, 1024], F32, tag="gh")
            # engine for the G-relu per group: spread across GPSIMD/ACT/DVE
            for g in range(32):
                b32 = g // 8
                pat = g % 8
                nkk4 = pp.tile([128, 1024], F32, tag="wide")
                for b in range(2):
                    nc.tensor.matmul(nkk4[:, 512 * b:512 * (b + 1)],
                                     lhsT=bc32_s[32 * b32:32 * (b32 + 1), pat, :],
                                     rhs=nkk16[32 * b32:32 * (b32 + 1), 512 * b:512 * (b + 1)],
                                     start=True, stop=True,
                                     tile_position=(32 * b32, 0))
                uh = sp2.tile([128, 1024], F32, tag="uh")
                nc.vector.scalar_tensor_tensor(uh[:], nkk4[:], aH4_s[:], T24[:],
                                               op0=OP.mult, op1=OP.add)
                ph = sp2.tile([128, 1024], F16, tag="ph")
                nc.scalar.activation(ph[:], uh[:], AF.Relu, bias=T14[:, g:g + 1])
                pg = sp2.tile([128, 1024], F16, tag="pg")
                if g % 4 == 3:
                    nc.vector.tensor_scalar(pg[:], B4[:], AT4[:, g:g + 1], 0.0, OP.add, OP.max)
                else:
                    nc.gpsimd.tensor_scalar(pg[:], B4[:], AT4[:, g:g + 1], 0.0, OP.add, OP.max)
                off = 124 - 4 * g
                for b in range(2):
                    nc.tensor.matmul(hin[:, 512 * b:512 * (b + 1)],
                                     lhsT=cch_s[:, off:off + 128],
                                     rhs=ph[:, 512 * b:512 * (b + 1)],
                                     start=(g == 0), stop=(g == 31))
                    nc.tensor.matmul(gin[:, 512 * b:512 * (b + 1)],
                                     lhsT=ccg_s[:, off:off + 128],
                                     rhs=pg[:, 512 * b:512 * (b + 1)],
                                     start=(g == 0), stop=(g == 31))

            # ---------------- tail: R, softmax, attention, FFN, LN ----------------
            rg = sp.tile([128, 1024], F32, tag="rg")
            nc.scalar.activation(rg[:], gin[:], AF.Relu, bias=g2b_s[:])
            rh = sp.tile([128, 1024], F32, tag="rh")
            nc.vector.tensor_scalar(rh[:], hin[:], h2b_s[:], 0.0, OP.add, OP.max)
            lg = sp.tile([128, 1024], F32, tag="lg")
            nc.vector.tensor_mul(lg[:], lraw[:], rg[:])
            nc.vector.tensor_mul(lg[:], lg[:], rh[:])
            mx = sp.tile([128, 1], F32, tag="mx")
            nc.vector.tensor_reduce(mx[:], lg[:], mybir.AxisListType.X, OP.max)
            nmx = sp.tile([128, 1], F32, tag="nmx")
            nc.vector.tensor_scalar_mul(nmx[:], mx[:], -1.0)
            pexp = sp.tile([128, 1024], F16, tag="pexp")
            sume = sp.tile([128, 1], F32, tag="sume")
            nc.scalar.activation(pexp[:], lg[:], AF.Exp, bias=nmx[:], accum_out=sume[:])
            rsum = sp.tile([128, 1], F32, tag="rsum")
            nc.vector.reciprocal(rsum[:], sume[:])
            # transpose pexp -> (128, 8, 128)
            pT = sp.tile([128, 8, 128], F16, tag="pT")
            for t in range(8):
                ptp = pp.tile([128, 128], F16, tag="wide")
                nc.tensor.transpose(ptp[:], pexp[:, 128 * t:128 * (t + 1)], i16_s[:])
                nc.vector.tensor_copy(pT[:, t, :], ptp[:])
            attn = pp.tile([128, 256], F32, tag="gh")
            for t in range(8):
                nc.tensor.matmul(attn[:], lhsT=pT[:, t, :], rhs=key3_s[:, t, :],
                                 start=(t == 0), stop=(t == 7))
            xpre = sp.tile([128, 256], F32, tag="xpre")
            nc.vector.scalar_tensor_tensor(xpre[:], attn[:], rsum[:], qsbh_s[:],
                                           op0=OP.mult, op1=OP.add)

            def layer_norm(src, gt, bt, tag):
                m = sp.tile([128, 1], F32, tag=f"m_{tag}")
                nc.vector.tensor_reduce(m[:], src[:], mybir.AxisListType.X, OP.add)
                nc.vector.tensor_scalar_mul(m[:], m[:], -1.0 / 256.0)
                xc = sp.tile([128, 256], F32, tag=f"xc_{tag}")
                nc.vector.tensor_scalar_add(xc[:], src[:], m[:])
                var = sp.tile([128, 1], F32, tag=f"v_{tag}")
                xc2 = sp.tile([128, 256], F32, tag=f"xc2_{tag}")
                nc.vector.scalar_tensor_tensor(xc2[:], xc[:], 1.0, xc[:],
                                               op0=OP.mult, op1=OP.mult, accum_out=var[:])
                lnv = sp.tile([128, 1], F32, tag=f"lv_{tag}")
                nc.scalar.activation(lnv[:], var[:], AF.Ln, scale=1.0 / 256.0, bias=eps_s[:])
                rstd = sp.tile([128, 1], F32, tag=f"rs_{tag}")
                nc.scalar.activation(rstd[:], lnv[:], AF.Exp, scale=-0.5)
                y = sp.tile([128, 256], F32, tag=f"y_{tag}")
                nc.vector.scalar_tensor_tensor(y[:], xc[:], rstd[:], gt[:],
                                               op0=OP.mult, op1=OP.mult)
                nc.vector.tensor_add(y[:], y[:], bt[:])
                return y

            x1 = layer_norm(xpre, l1g_s, l1b_s, "ln1")
            # FFN
            x1h = sp.tile([128, 256], F16, tag="x1h")
            nc.vector.tensor_copy(x1h[:], x1[:])
            xT = sp.tile([128, 2, 128], F16, tag="xT")
            for c in range(2):
                pxT = pp.tile([128, 128], F16, tag="wide")
                nc.tensor.transpose(pxT[:], x1h[:, 128 * c:128 * (c + 1)], i16_s[:])
                nc.vector.tensor_copy(xT[:, c, :], pxT[:])
            hT = sp.tile([128, 8, 128], F16, tag="hT")
            for t in range(8):
                psh = pp.tile([128, 128], F32, tag="wide")
                for c in range(2):
                    nc.tensor.matmul(psh[:], lhsT=f1w_s[:, c, t, :], rhs=xT[:, c, :],
                                     start=(c == 0), stop=(c == 1))
                nc.scalar.activation(hT[:, t, :], psh[:], AF.Relu, bias=f1b_s[:, t:t + 1])
            pso = pp.tile([128, 256], F32, tag="gh")
            for t in range(8):
                nc.tensor.matmul(pso[:], lhsT=hT[:, t, :], rhs=f2w_s[:, t, :],
                                 start=(t == 0), stop=(t == 7))
            y2 = sp.tile([128, 256], F32, tag="y2")
            nc.vector.tensor_add(y2[:], pso[:], x1[:])
            nc.vector.tensor_add(y2[:], y2[:], f2b_s[:])
            x2 = layer_norm(y2, l2g_s, l2b_s, "ln2")
            nc.sync.dma_start(out=out_d[:], in_=x2[:])

    _split_multiwaits(nc)
    return nc


# ---------------------------------------------------------------------------
def prep_inputs(inp):
    f32 = np.float32
    f16 = np.float16
    q_asn = np.asarray(inp["q_assignments"], f32)
    k_asn = np.asarray(inp["k_assignments"], f32)
    query = np.asarray(inp["query"], f32)
    key = np.asarray(inp["key_emb"], f32)

    def xfeat(coords, iso):
        oh = np.zeros((N, NISO), f32)
        oh[np.arange(N), np.asarray(iso) - 1] = 1.0
        x = np.concatenate([oh, np.ones((N, 1), f32), np.asarray(coords, f32)], axis=1)
        assert x.shape[1] == 15
        x = np.concatenate([x, np.zeros((N, 1), f32)], axis=1)  # pad to 16
        return np.ascontiguousarray(x.reshape(NCH, 128, XC).transpose(1, 0, 2)).astype(f16)

    qx_t = xfeat(inp["q_coords"], inp["q_iso"])
    kx_t = xfeat(inp["k_coords"], inp["k_iso"])

    g1 = np.asarray(inp["g1_w"], f32)          # (24, 32)
    g1b = np.asarray(inp["g1_b"], f32)         # (32,)
    g2 = np.asarray(inp["g2_w"], f32)[:, 0]    # (32,)
    g2b = float(np.asarray(inp["g2_b"], f32)[0])
    h1 = np.asarray(inp["h1_w"], f32)          # (3, 32)
    h1b = np.asarray(inp["h1_b"], f32)
    h2 = np.asarray(inp["h2_w"], f32)[:, 0]
    h2b = float(np.asarray(inp["h2_b"], f32)[0])

    aw2 = np.abs(g2)
    sg2 = np.sign(g2).astype(f32)
    aw2h = np.abs(h2)
    sh2 = np.sign(h2).astype(f32)

    g1ab = np.concatenate([g1[:12] * aw2[None, :], (g1b * aw2)[None, :]], axis=0)  # (13,32)
    g1b4 = np.tile(g1[12:] * aw2[None, :], (1, 4))                                  # (12,128)

    def ccpat(sgn):
        cc = np.zeros((128, 256), f32)
        for ii in range(4):
            for c in range(32):
                cc[32 * ii + c, 124 + ii] = sgn[c]
        return cc.astype(f16)

    cc_g = ccpat(sg2)
    cc_h = ccpat(sh2)

    bc32 = np.zeros((32, 8, 128), f32)
    for pat in range(8):
        for ii in range(4):
            for c in range(32):
                bc32[4 * pat + ii, pat, 32 * ii + c] = 1.0
    bc32 = np.tile(bc32, (4, 1, 1)).astype(f16)

    wq = (h1[1] * aw2h)[None, :].astype(f32)       # (1,32)
    b1r = (h1b * aw2h)[None, :].astype(f32)
    wk4 = np.tile(h1[2] * aw2h, 4)[None, :].astype(f32)   # (1,128)
    aH4 = np.tile(h1[0] * aw2h, 4)[:, None].astype(f32)   # (128,1)

    sens = np.asarray(inp["sensor_coords"], f32)[None, :]          # (1,2)
    sensr = np.tile(sens, (128, 1))                                 # (128,2)

    f1wt = np.ascontiguousarray(
        np.asarray(inp["ffn1_w"], f32).reshape(2, 128, 8, 128).transpose(1, 0, 2, 3)).astype(f16)
    f1b = np.ascontiguousarray(np.asarray(inp["ffn1_b"], f32).reshape(8, 128).T)
    f2wt = np.ascontiguousarray(
        np.asarray(inp["ffn2_w"], f32).reshape(8, 128, 256).transpose(1, 0, 2)).astype(f16)
    f2br = np.tile(np.asarray(inp["ffn2_b"], f32)[None, :], (128, 1))
    l1g = np.tile(np.asarray(inp["ln1_g"], f32)[None, :], (128, 1))
    l1b = np.tile(np.asarray(inp["ln1_b"], f32)[None, :], (128, 1))
    l2g = np.tile(np.asarray(inp["ln2_g"], f32)[None, :], (128, 1))
    l2b = np.tile(np.asarray(inp["ln2_b"], f32)[None, :], (128, 1))

    keyT3 = np.ascontiguousarray(key.T.reshape(2, 128, 1024).transpose(1, 0, 2)).astype(f16)
    key3 = np.ascontiguousarray(key.reshape(8, 128, 256).transpose(1, 0, 2)).astype(f16)

    shared = {
        "qx_t": qx_t, "kx_t": kx_t,
        "keyT3": keyT3, "key3": key3,
        "g1ab": g1ab, "g1b4": g1b4, "cc_g": cc_g, "cc_h": cc_h, "bc32": bc32,
        "wq_h": wq, "b1_h": b1r, "wk4": wk4, "aH4": aH4,
        "g2b": np.full((128, 1), g2b, f32), "h2b": np.full((128, 1), h2b, f32),
        "ones_r": np.ones((1, 128), f32),
        "sens": sens, "sensr": sensr,
        "i128f": np.eye(128, dtype=f32), "i128h": np.eye(128, dtype=f16),
        "f1w": f1wt, "f1b": f1b, "f2w": f2wt, "f2br": f2br,
        "l1g": l1g, "l1b": l1b, "l2g": l2g, "l2b": l2b,
        "epsc": np.full((128, 1), 1e-6, f32),
    }

    in_maps = []
    for m in range(NCORES):
        sl = slice(m * KSL, (m + 1) * KSL)
        qa = np.ascontiguousarray(
            q_asn[:, sl].reshape(NCH, 128, 128).transpose(1, 0, 2))
        ka = np.ascontiguousarray(
            k_asn[:, sl].reshape(NCH, 128, 128).transpose(1, 0, 2))
        qT3 = np.ascontiguousarray(
            query[sl].T.reshape(2, 128, 128).transpose(1, 0, 2)).astype(f16)
        im = dict(shared)
        im.update({
            "qa_t": qa, "ka_t": ka, "qT3": qT3,
            "q_sbh": np.ascontiguousarray(query[sl]),
        })
        in_maps.append(im)
    return in_maps


def kernel(**inputs) -> np.ndarray:
    if "nc" not in _cache:
        _cache["nc"] = build_program()
    nc = _cache["nc"]
    in_maps = prep_inputs(inputs)
    res = run_bass_kernel_spmd(nc, in_maps, list(range(NCORES)))
    return np.concatenate([res.results[m]["out"] for m in range(NCORES)], axis=0)



# revision 4
# speedup vs baseline: 1.3576x; 1.3576x over previous
"""Trainium2 Bass kernel for nn_CustomAttentionLayer (sparse_attention).

Strategy (8 NeuronCores, SPMD single launch), v2:
 - Shard the K=1024 query-cluster axis: core m owns rows [128m, 128m+128).
 - Phase 1 (DMA-bound): assignments are host-cast to fp8e4 (scaled x256) and
   streamed through the PE with the matmul oriented so the SMALL feature
   matrix X = [one_hot(iso) | ones | coords] (fp8) is the moving operand:
   out psum = A_chunk.T @ X_chunk costs only 16 PE cycles per 128-pixel
   chunk. k-side reduced first, AllGather'd (8KB) while the q-side streams.
   The q-side is streamed in two 64-column halves so phase 2 can start on
   half 0 while half 1 is still in flight.
 - Phase 2: R = G*H with (4i x 32c) partition packing. H hidden pre-act is
   built directly on the PE from an interleaved V tile
   [nkk(4 rows)|n_kps] x 16 groups via a constant (5,128) lhsT; relu comes
   off PSUM on ACT/DVE with the per-partition T14 bias; G hidden is
   relu(B4 + AT4 col) on Pool/ACT/DVE. Hidden activations are written as
   fp8e4 and contracted over the channel axis in group-pairs with
   DoubleRow fp8 matmuls accumulating into (128,1024) G/H psums.
 - Tail: logits*R, softmax (DMA-transpose), attention, FFN, layernorms.
"""
import numpy as np
import ml_dtypes

import concourse.bass as bass
import concourse.mybir as mybir
import concourse.tile as tile
from concourse.bass_utils import run_bass_kernel_spmd

F32 = mybir.dt.float32
F16 = mybir.dt.float16
F8 = mybir.dt.float8e4
AF = mybir.ActivationFunctionType
OP = mybir.AluOpType
DR = mybir.MatmulPerfMode.DoubleRow
NP8 = ml_dtypes.float8_e4m3

NCORES = 8
K, D, N, NISO = 1024, 256, 65536, 12
KSL = K // NCORES           # 128 rows per core
NCH = N // 128              # 512 contraction chunks
GRP = 64                    # chunks per DMA group
NGRP = NCH // GRP           # 8
XC = 16                     # X feature cols: [oh(12) | ones | cx | cy | 0]
ASCALE = 256.0              # host-side scale on assignments before fp8 cast
NH = 2                      # q-side halves
HW = KSL // NH              # 64 i-rows per half
HG = 16                     # phase-2 groups per half

_cache = {}


# ---------------------------------------------------------------------------
# walrus in this container rejects >1 sync wait per instruction; split extras
# onto single-wait NOPs on the same engine right before the instruction.
def _split_multiwaits(nc):
    ctr = 0
    for f in nc.m.functions:
        for bb in f.blocks:
            for inst in list(bb.instructions):
                si = inst.sync_info
                if si is None:
                    continue
                waits = list(si.on_wait)
                if len(waits) <= 1:
                    continue
                si.on_wait = [waits[-1]]
                pos = None
                for j, cur in enumerate(bb.instructions):
                    if cur.name == inst.name:
                        pos = j
                        break
                assert pos is not None
                for k2, w in enumerate(waits[:-1]):
                    nop = mybir.InstNoOp(
                        name=f"wsplit-{ctr}",
                        sync_info=mybir.SyncInfo(on_wait=[w], on_update=[]),
                        engine=inst.engine,
                        bass_nofuse=True,
                    )
                    ctr += 1
                    nc.register_instruction(nop)
                    bb.instructions.insert(pos + k2, nop)
    return ctr


def build_program():
    nc = bass.Bass()

    def din(name, shape, dt=F32):
        return nc.dram_tensor(name, list(shape), dt, kind="ExternalInput")

    # big streams
    ka8_h = din("ka8", (128, NCH, 128), F8)        # k-assignments slice, chunked
    qa8_h = din("qa8", (128, NH, NCH, HW), F8)     # q-assignments slice, halved
    kx8_h = din("kx8", (128, NCH, XC), F8)         # k-side features
    qx8_h = din("qx8", (128, NCH, XC), F8)
    # attention / ffn consts
    qT3_h = din("qT3", (128, 2, 128), F16)         # query[sl].T chunked
    keyT3_h = din("keyT3", (128, 2, 1024), F16)    # key.T chunked
    key3_h = din("key3", (128, 8, 256), F16)       # key chunked
    q_sb_h = din("q_sbh", (128, 256))              # query[sl] fp32
    f1w_h = din("f1w", (128, 2, 8, 128), F16)
    f1b_h = din("f1b", (128, 8))
    f2w_h = din("f2w", (128, 8, 256), F16)
    f2b_h = din("f2br", (128, 256))
    l1g_h = din("l1g", (128, 256))
    l1b_h = din("l1b", (128, 256))
    l2g_h = din("l2g", (128, 256))
    l2b_h = din("l2b", (128, 256))
    eps_h = din("epsc", (128, 1))
    # G / H mlp packing
    g1ab_h = din("g1ab", (13, 32))                 # [G1A*|g2| ; g1_b*|g2|]
    g1b4_h = din("g1b4", (12, 128), F16)           # G1B*|g2| tiled x4
    hpatV_h = din("hpatV", (80, HG, 128), F16)     # H-build lhsT, zero-padded per group
    wq_h = din("wq_h", (1, 32))                    # h1_w[1]*|h2| (f32)
    b1_h = din("b1_h", (1, 32))                    # h1_b*|h2|
    g2b_h = din("g2b", (128, 1))
    h2b_h = din("h2b", (128, 1))
    ccgDR_h = din("ccgDR", (128, HG // 2 * NH, 2, 128), F8)  # G contract DR pats
    cchDR_h = din("cchDR", (128, HG // 2 * NH, 2, 128), F8)
    permV_h = din("permV", (HW, 80), F16)          # nkk row -> V row perm
    insV_h = din("insV", (1, 80), F16)             # nkps row insert
    ones16_h = din("ones16", (1, 128), F16)
    ones32_h = din("ones32", (1, 64))              # f32 ones row
    i64f_h = din("i64f", (64, 64))                 # f32 identity (transposes)
    i128f_h = din("i128f", (128, 128))
    sens_h = din("sens", (1, 2))
    sensr_h = din("sensr", (64, 2))

    out_d = nc.dram_tensor("out", [128, 256], F32, kind="ExternalOutput")
    kside_d = nc.dram_tensor("kside", [16, 128], F32)
    kall_d = nc.dram_tensor("kall", [128, 128], F32, addr_space="Shared")

    with tile.TileContext(nc) as tc:
        with (
            tc.tile_pool(name="consts", bufs=1) as cp,
            tc.tile_pool(name="astream", bufs=3) as ap,
            tc.tile_pool(name="sb", bufs=1) as sp,
            tc.tile_pool(name="ppP", bufs=1, space="PSUM") as ppP,   # gin/hin
            tc.tile_pool(name="ppH", bufs=2, space="PSUM") as ppH,   # H-build
            tc.tile_pool(name="ppM", bufs=2, space="PSUM") as ppM,   # misc
        ):
            def cload(dram, eng=None, tag=None):
                t = cp.tile(list(dram.shape), dram.dtype, tag=tag or dram.name)
                (eng or nc.sync).dma_start(out=t[:], in_=dram[:])
                return t

            # X features first (k-reduce needs kx), via SP queue
            kx_s = cload(kx8_h)
            qx_s = cload(qx8_h)
            # everything else on the scalar (ACT) queue so SP streams A
            sc = nc.scalar
            qT3_s = cload(qT3_h, sc)
            keyT3_s = cload(keyT3_h, sc)
            key3_s = cload(key3_h, sc)
            qsbh_s = cload(q_sb_h, sc)
            f1w_s = cload(f1w_h, sc)
            f1b_s = cload(f1b_h, sc)
            f2w_s = cload(f2w_h, sc)
            f2b_s = cload(f2b_h, sc)
            l1g_s = cload(l1g_h, sc)
            l1b_s = cload(l1b_h, sc)
            l2g_s = cload(l2g_h, sc)
            l2b_s = cload(l2b_h, sc)
            eps_s = cload(eps_h, sc)
            g1ab_s = cload(g1ab_h, sc)
            g1b4_s = cload(g1b4_h, sc)
            hpatV_s = cload(hpatV_h, sc)
            wq_s = cload(wq_h, sc)
            b1_s = cload(b1_h, sc)
            g2b_s = cload(g2b_h, sc)
            h2b_s = cload(h2b_h, sc)
            ccg_s = cload(ccgDR_h, sc)
            cch_s = cload(cchDR_h, sc)
            permV_s = cload(permV_h, sc)
            insV_s = cload(insV_h, sc)
            ones16_s = cload(ones16_h, sc)
            ones32_s = cload(ones32_h, sc)
            i64f_s = cload(i64f_h, sc)
            i128f_s = cload(i128f_h, sc)
            sens_s = cload(sens_h, sc)
            sensr_s = cload(sensr_h, sc)

            # ---------------- phase 1: k-side reduction ----------------
            psk = ppH.tile([128, XC], F32, tag="psH")
            for g in range(NGRP):
                at = ap.tile([128, GRP, 128], F8, tag="ka")
                nc.sync.dma_start(out=at[:], in_=ka8_h[:, g * GRP:(g + 1) * GRP, :])
                for c in range(GRP):
                    nc.tensor.matmul(
                        psk[:], lhsT=at[:, c, :], rhs=kx_s[:, g * GRP + c, :],
                        start=(g == 0 and c == 0),
                        stop=(g == NGRP - 1 and c == GRP - 1),
                    )
            # transpose (128,16) -> (16,128) and ship to the collective
            ksb = sp.tile([128, XC], F32, tag="ksb")
            nc.vector.tensor_copy(ksb[:], psk[:])
            pskT = ppM.tile([XC, 128], F32, tag="m")
            nc.tensor.transpose(pskT[:], ksb[:], i128f_s[:])
            ksbT = sp.tile([XC, 128], F32, tag="ksbT")
            nc.vector.tensor_copy(ksbT[:], pskT[:])
            nc.scalar.dma_start(out=kside_d[:], in_=ksbT[:])
            nc.gpsimd.collective_compute(
                "AllGather", OP.bypass,
                replica_groups=[list(range(NCORES))],
                ins=[kside_d[:]],
                outs=[kall_d[:]],
            )
            kview = kall_d.rearrange("(g c) k -> c g k", c=16)
            dkpT = sp.tile([12, 1024], F32, tag="dkpT")
            nc.scalar.dma_start(out=dkpT[:].rearrange("c (g k) -> c g k", g=8),
                              in_=kview[0:12, :, :])
            ksum_r = sp.tile([1, 1024], F32, tag="ksum_r")
            nc.scalar.dma_start(out=ksum_r[:].rearrange("c (g k) -> c g k", g=8),
                              in_=kview[12:13, :, :])
            kcxs_r = sp.tile([1, 1024], F32, tag="kcxs_r")
            nc.scalar.dma_start(out=kcxs_r[:].rearrange("c (g k) -> c g k", g=8),
                              in_=kview[13:14, :, :])
            kcys_r = sp.tile([1, 1024], F32, tag="kcys_r")
            nc.scalar.dma_start(out=kcys_r[:].rearrange("c (g k) -> c g k", g=8),
                              in_=kview[14:15, :, :])

            # ---------------- q-side streaming (both halves) ----------------
            psq = []
            for h in range(NH):
                ps = ppH.tile([HW, XC], F32, tag="psH")
                for g in range(NGRP):
                    at = ap.tile([128, GRP, HW], F8, tag="qa")
                    nc.sync.dma_start(out=at[:], in_=qa8_h[:, h, g * GRP:(g + 1) * GRP, :])
                    for c in range(GRP):
                        nc.tensor.matmul(
                            ps[:], lhsT=at[:, c, :], rhs=qx_s[:, g * GRP + c, :],
                            start=(g == 0 and c == 0),
                            stop=(g == NGRP - 1 and c == GRP - 1),
                        )
                psq.append(ps)

            # ---------------- logits (consts only; runs under the stream) ----
            lraw = sp.tile([128, 1024], F16, tag="lraw")
            for b in range(2):
                psl = ppM.tile([128, 512], F32, tag="m")
                for c in range(2):
                    nc.tensor.matmul(psl[:], lhsT=qT3_s[:, c, :],
                                     rhs=keyT3_s[:, c, 512 * b:512 * (b + 1)],
                                     start=(c == 0), stop=(c == 1))
                nc.scalar.activation(lraw[:, 512 * b:512 * (b + 1)], psl[:],
                                     AF.Copy, scale=1.0 / 16.0)

            # ---------------- k-side prep (after collective) ----------------
            rk_r = sp.tile([1, 1024], F32, tag="rk_r")
            nc.vector.tensor_scalar_add(rk_r[:], ksum_r[:], ASCALE * 1e-6)
            nc.vector.reciprocal(rk_r[:], rk_r[:])
            kcx_r = sp.tile([1, 1024], F32, tag="kcx_r")
            nc.vector.tensor_mul(kcx_r[:], kcxs_r[:], rk_r[:])
            kcy_r = sp.tile([1, 1024], F32, tag="kcy_r")
            nc.vector.tensor_mul(kcy_r[:], kcys_r[:], rk_r[:])
            kcx16 = sp.tile([1, 1024], F16, tag="kcx16")
            nc.vector.tensor_copy(kcx16[:], kcx_r[:])
            kcy16 = sp.tile([1, 1024], F16, tag="kcy16")
            nc.vector.tensor_copy(kcy16[:], kcy_r[:])
            # n_kps row
            s1 = sp.tile([1, 1024], F32, tag="s1")
            s2 = sp.tile([1, 1024], F32, tag="s2")
            nc.vector.tensor_scalar_sub(s1[:], kcx_r[:], sens_s[0:1, 0:1])
            nc.vector.tensor_scalar_sub(s2[:], kcy_r[:], sens_s[0:1, 1:2])
            nc.vector.tensor_mul(s1[:], s1[:], s1[:])
            nc.vector.tensor_mul(s2[:], s2[:], s2[:])
            nc.vector.tensor_add(s1[:], s1[:], s2[:])
            nkps16 = sp.tile([1, 1024], F16, tag="nkps16")
            nc.scalar.activation(nkps16[:], s1[:], AF.Sqrt)
            # dkp normalized (12,1024) then B4 = g1b4.T @ dkpn  (f16, halves)
            dkpn = sp.tile([12, 1024], F16, tag="dkpn")
            B4 = sp.tile([128, 1024], F16, tag="B4")
            for b in range(2):
                sl = slice(512 * b, 512 * (b + 1))
                rkb = ppM.tile([12, 512], F32, tag="m")
                nc.tensor.matmul(rkb[:], lhsT=ones32_s[0:1, 0:12], rhs=rk_r[0:1, sl],
                                 start=True, stop=True)
                nc.vector.tensor_mul(dkpn[:, sl], dkpT[:, sl], rkb[:])
                psB = ppM.tile([128, 512], F32, tag="m")
                nc.tensor.matmul(psB[:], lhsT=g1b4_s[:], rhs=dkpn[:, sl],
                                 start=True, stop=True)
                nc.scalar.activation(B4[:, sl], psB[:], AF.Copy)

            # ---------------- per-half q prep / nkk / V / phase 2 ----------
            gin = ppP.tile([128, 1024], F32, tag="gin")
            hin = ppP.tile([128, 1024], F32, tag="hin")
            ph2 = sp.tile([128, 2, 1024], F8, tag="ph2")
            pg2 = sp.tile([128, 2, 1024], F8, tag="pg2")

            # engine schedules for the relus (per half-group index 0..15)
            g_eng = [nc.gpsimd] * 9 + [nc.scalar] * 2 + [nc.vector] * 5
            hlo_eng = ([nc.scalar] * 5 + [nc.vector] * 3) * 2
            hhi_eng = ([nc.vector] * 3 + [nc.scalar] * 5) * 2

            def q_half(h):
                """q-side stats for half h -> (AT4h, T14h, nqc2, V80h)."""
                qT = sp.tile([HW, XC], F32, tag=f"qT{h}")
                nc.vector.tensor_copy(qT[:], psq[h][:])
                rq = sp.tile([HW, 1], F32, tag=f"rq{h}")
                nc.vector.tensor_scalar_add(rq[:], qT[:, 12:13], ASCALE * 1e-6)
                nc.vector.reciprocal(rq[:], rq[:])
                qn = sp.tile([HW, 13], F32, tag=f"qn{h}")
                nc.vector.tensor_scalar_mul(qn[:], qT[:, 0:13], rq[:])
                qc2 = sp.tile([HW, 2], F32, tag=f"qc2{h}")
                nc.vector.tensor_scalar_mul(qc2[:], qT[:, 13:15], rq[:])
                nqc2 = sp.tile([HW, 2], F32, tag=f"nqc2{h}")
                nc.vector.tensor_scalar_mul(nqc2[:], qc2[:], -1.0)
                d2 = sp.tile([HW, 2], F32, tag=f"d2{h}")
                nc.vector.tensor_sub(d2[:], qc2[:], sensr_s[:])
                nc.vector.tensor_mul(d2[:], d2[:], d2[:])
                nks_c = sp.tile([HW, 1], F32, tag=f"nksc{h}")
                nc.vector.tensor_reduce(nks_c[:], d2[:], mybir.AxisListType.X, OP.add)
                nc.scalar.activation(nks_c[:], nks_c[:], AF.Sqrt)
                # transposes
                pqnT = ppM.tile([13, HW], F32, tag="m")
                nc.tensor.transpose(pqnT[:], qn[:], i64f_s[:])
                qsT = sp.tile([13, HW], F32, tag=f"qsT{h}")
                nc.vector.tensor_copy(qsT[:], pqnT[:])
                pnksT = ppM.tile([1, HW], F32, tag="m")
                nc.tensor.transpose(pnksT[:], nks_c[:], i64f_s[:])
                nks_r = sp.tile([1, HW], F32, tag=f"nksr{h}")
                nc.vector.tensor_copy(nks_r[:], pnksT[:])
                # AT4h (128, 16): packed A'' bias for this half's 16 groups
                psA = ppM.tile([128, HG], F32, tag="m")
                for ii in range(4):
                    nc.tensor.matmul(psA[32 * ii:32 * (ii + 1), :], lhsT=g1ab_s[:],
                                     rhs=qsT[:, ii::4], start=True, stop=True,
                                     tile_position=(0, 32 * ii))
                AT4h = sp.tile([128, HG], F32, tag=f"AT4{h}")
                nc.vector.tensor_copy(AT4h[:], psA[:])
                # T14h (128, 16): packed H bias
                psT1 = ppM.tile([128, HG], F32, tag="m")
                for ii in range(4):
                    nc.tensor.matmul(psT1[32 * ii:32 * (ii + 1), :], lhsT=wq_s[:],
                                     rhs=nks_r[0:1, ii::4], start=True, stop=False,
                                     tile_position=(0, 32 * ii))
                    nc.tensor.matmul(psT1[32 * ii:32 * (ii + 1), :], lhsT=b1_s[:],
                                     rhs=ones32_s[0:1, ii::4], start=False, stop=True,
                                     tile_position=(0, 32 * ii))
                T14h = sp.tile([128, HG], F32, tag=f"T14{h}")
                nc.vector.tensor_copy(T14h[:], psT1[:])
                # nkk (HW,1024) f16 via single-m-bank rounds, then V80
                dx2 = sp.tile([HW, 1024], F16, tag=f"dx2{h}")
                dy2 = sp.tile([HW, 1024], F16, tag=f"dy2{h}")
                for b in range(2):
                    sl = slice(512 * b, 512 * (b + 1))
                    pK = ppM.tile([HW, 512], F32, tag="m")
                    nc.tensor.matmul(pK[:], lhsT=ones16_s[0:1, 0:HW], rhs=kcx16[0:1, sl],
                                     start=True, stop=True)
                    nc.scalar.activation(dx2[:, sl], pK[:], AF.Square,
                                         bias=nqc2[:, 0:1])
                    pK2 = ppM.tile([HW, 512], F32, tag="m")
                    nc.tensor.matmul(pK2[:], lhsT=ones16_s[0:1, 0:HW], rhs=kcy16[0:1, sl],
                                     start=True, stop=True)
                    nc.scalar.activation(dy2[:, sl], pK2[:], AF.Square,
                                         bias=nqc2[:, 1:2])
                nc.vector.tensor_add(dx2[:], dx2[:], dy2[:])
                nkk = sp.tile([HW, 1024], F16, tag=f"nkk{h}")
                nc.scalar.activation(nkk[:], dx2[:], AF.Sqrt)
                # V80: interleaved [nkk x4 | nkps] x 16 groups
                V80 = sp.tile([80, 1024], F16, tag=f"V80{h}")
                for b in range(2):
                    sl = slice(512 * b, 512 * (b + 1))
                    pV = ppM.tile([80, 512], F32, tag="m")
                    nc.tensor.matmul(pV[:], lhsT=permV_s[:], rhs=nkk[:, sl],
                                     start=True, stop=False)
                    nc.tensor.matmul(pV[:], lhsT=insV_s[:], rhs=nkps16[0:1, sl],
                                     start=False, stop=True)
                    nc.scalar.activation(V80[:, sl], pV[:], AF.Copy)
                return AT4h, T14h, V80

            def phase2_half(h, AT4h, T14h, V80):
                for gp in range(HG):
                    g = HG * h + gp
                    s = g & 1
                    # H-build in two 512-wide psum banks
                    psa = ppH.tile([128, 512], F32, tag="psH")
                    nc.tensor.matmul(psa[:], lhsT=hpatV_s[:, gp, :],
                                     rhs=V80[:, 0:512], start=True, stop=True)
                    psb = ppH.tile([128, 512], F32, tag="psH")
                    nc.tensor.matmul(psb[:], lhsT=hpatV_s[:, gp, :],
                                     rhs=V80[:, 512:1024], start=True, stop=True)
                    def _relu(eng, dst, src, bcol):
                        if eng is nc.scalar:
                            nc.scalar.activation(dst, src, AF.Relu, bias=bcol)
                        else:
                            eng.tensor_scalar(dst, src, bcol, 0.0, OP.add, OP.max)
                    _relu(hlo_eng[gp], ph2[:, s, 0:512], psa[:], T14h[:, gp:gp + 1])
                    _relu(hhi_eng[gp], ph2[:, s, 512:1024], psb[:], T14h[:, gp:gp + 1])
                    _relu(g_eng[gp], pg2[:, s, :], B4[:], AT4h[:, gp:gp + 1])
                    if s == 1:
                        pr = g // 2
                        for b in range(2):
                            sl = slice(512 * b, 512 * (b + 1))
                            nc.tensor.matmul(hin[:, sl], lhsT=cch_s[:, pr, :, :],
                                             rhs=ph2[:, :, sl], perf_mode=DR,
                                             start=(pr == 0), stop=(pr == 15))
                            nc.tensor.matmul(gin[:, sl], lhsT=ccg_s[:, pr, :, :],
                                             rhs=pg2[:, :, sl], perf_mode=DR,
                                             start=(pr == 0), stop=(pr == 15))

            AT40, T140, V800 = q_half(0)
            phase2_half(0, AT40, T140, V800)
            AT41, T141, V801 = q_half(1)
            phase2_half(1, AT41, T141, V801)

            # ---------------- tail: R, softmax, attention, FFN, LN ----------
            rg = sp.tile([128, 1024], F16, tag="rg")
            nc.scalar.activation(rg[:], gin[:], AF.Relu, bias=g2b_s[:])
            rh = sp.tile([128, 1024], F16, tag="rh")
            nc.vector.tensor_scalar(rh[:], hin[:], h2b_s[:], 0.0, OP.add, OP.max)
            lg = sp.tile([128, 1024], F16, tag="lg")
            nc.vector.tensor_mul(lg[:], rg[:], rh[:])
            nc.vector.tensor_mul(lg[:], lg[:], lraw[:])
            mx = sp.tile([128, 1], F32, tag="mx")
            nc.vector.tensor_reduce(mx[:], lg[:], mybir.AxisListType.X, OP.max)
            nmx = sp.tile([128, 1], F32, tag="nmx")
            nc.vector.tensor_scalar_mul(nmx[:], mx[:], -1.0)
            pexp = sp.tile([128, 1024], F16, tag="pexp")
            sume = sp.tile([128, 1], F32, tag="sume")
            nc.scalar.activation(pexp[:], lg[:], AF.Exp, bias=nmx[:], accum_out=sume[:])
            rsum = sp.tile([128, 1], F32, tag="rsum")
            nc.vector.reciprocal(rsum[:], sume[:])
            # transpose pexp -> (128, 8, 128) via DMA transpose (off-engine)
            pT = sp.tile([128, 8, 128], F16, tag="pT")
            nc.scalar.dma_start_transpose(pT[:], pexp[:])
            attn = ppM.tile([128, 256], F32, tag="m")
            for t in range(8):
                nc.tensor.matmul(attn[:], lhsT=pT[:, t, :], rhs=key3_s[:, t, :],
                                 start=(t == 0), stop=(t == 7))
            xpre = sp.tile([128, 256], F32, tag="xpre")
            nc.vector.scalar_tensor_tensor(xpre[:], attn[:], rsum[:], qsbh_s[:],
                                           op0=OP.mult, op1=OP.add)

            def layer_norm(src, gt, bt, tag):
                m = sp.tile([128, 1], F32, tag=f"m_{tag}")
                nc.vector.tensor_reduce(m[:], src[:], mybir.AxisListType.X, OP.add)
                nc.vector.tensor_scalar_mul(m[:], m[:], -1.0 / 256.0)
                xc = sp.tile([128, 256], F32, tag=f"xc_{tag}")
                nc.vector.tensor_scalar_add(xc[:], src[:], m[:])
                var = sp.tile([128, 1], F32, tag=f"v_{tag}")
                xc2 = sp.tile([128, 256], F32, tag=f"xc2_{tag}")
                nc.vector.scalar_tensor_tensor(xc2[:], xc[:], 1.0, xc[:],
                                               op0=OP.mult, op1=OP.mult, accum_out=var[:])
                lnv = sp.tile([128, 1], F32, tag=f"lv_{tag}")
                nc.scalar.activation(lnv[:], var[:], AF.Ln, scale=1.0 / 256.0, bias=eps_s[:])
                rstd = sp.tile([128, 1], F32, tag=f"rs_{tag}")
                nc.scalar.activation(rstd[:], lnv[:], AF.Exp, scale=-0.5)
                y = sp.tile([128, 256], F32, tag=f"y_{tag}")
                nc.vector.scalar_tensor_tensor(y[:], xc[:], rstd[:], gt[:],
                                               op0=OP.mult, op1=OP.mult)
                nc.vector.tensor_add(y[:], y[:], bt[:])
                return y

            x1 = layer_norm(xpre, l1g_s, l1b_s, "ln1")
            # FFN
            x1h = sp.tile([128, 256], F16, tag="x1h")
            nc.vector.tensor_copy(x1h[:], x1[:])
            xT = sp.tile([128, 2, 128], F16, tag="xT")
            nc.scalar.dma_start_transpose(xT[:], x1h[:])
            hT = sp.tile([128, 8, 128], F16, tag="hT")
            for t in range(8):
                psh = ppM.tile([128, 128], F32, tag="m")
                for c in range(2):
                    nc.tensor.matmul(psh[:], lhsT=f1w_s[:, c, t, :], rhs=xT[:, c, :],
                                     start=(c == 0), stop=(c == 1))
                if t % 2 == 0:
                    nc.scalar.activation(hT[:, t, :], psh[:], AF.Relu, bias=f1b_s[:, t:t + 1])
                else:
                    nc.vector.tensor_scalar(hT[:, t, :], psh[:], f1b_s[:, t:t + 1],
                                            0.0, OP.add, OP.max)
            pso = ppM.tile([128, 256], F32, tag="m")
            for t in range(8):
                nc.tensor.matmul(pso[:], lhsT=hT[:, t, :], rhs=f2w_s[:, t, :],
                                 start=(t == 0), stop=(t == 7))
            y2 = sp.tile([128, 256], F32, tag="y2")
            nc.vector.tensor_add(y2[:], pso[:], x1[:])
            nc.vector.tensor_add(y2[:], y2[:], f2b_s[:])
            x2 = layer_norm(y2, l2g_s, l2b_s, "ln2")
            nc.scalar.dma_start(out=out_d[:], in_=x2[:])

    _split_multiwaits(nc)
    return nc


# ---------------------------------------------------------------------------
def prep_inputs(inp):
    f32 = np.float32
    f16 = np.float16
    q_asn = np.asarray(inp["q_assignments"], f32)
    k_asn = np.asarray(inp["k_assignments"], f32)
    query = np.asarray(inp["query"], f32)
    key = np.asarray(inp["key_emb"], f32)

    def xfeat(coords, iso):
        oh = np.zeros((N, NISO), f32)
        oh[np.arange(N), np.asarray(iso) - 1] = 1.0
        x = np.concatenate([oh, np.ones((N, 1), f32), np.asarray(coords, f32),
                            np.zeros((N, 1), f32)], axis=1)
        return np.ascontiguousarray(
            x.reshape(NCH, 128, XC).transpose(1, 0, 2)).astype(NP8)

    qx8 = xfeat(inp["q_coords"], inp["q_iso"])
    kx8 = xfeat(inp["k_coords"], inp["k_iso"])

    g1 = np.asarray(inp["g1_w"], f32)          # (24, 32)
    g1b = np.asarray(inp["g1_b"], f32)         # (32,)
    g2 = np.asarray(inp["g2_w"], f32)[:, 0]    # (32,)
    g2b = float(np.asarray(inp["g2_b"], f32)[0])
    h1 = np.asarray(inp["h1_w"], f32)          # (3, 32)
    h1b = np.asarray(inp["h1_b"], f32)
    h2 = np.asarray(inp["h2_w"], f32)[:, 0]
    h2b = float(np.asarray(inp["h2_b"], f32)[0])

    ag, sg = np.abs(g2), np.sign(g2).astype(f32)
    ah, sh = np.abs(h2), np.sign(h2).astype(f32)

    g1ab = np.concatenate([g1[:12] * ag[None, :], (g1b * ag)[None, :]], axis=0)  # (13,32)
    g1b4 = np.tile(g1[12:] * ag[None, :], (1, 4))                                # (12,128)

    hpatV = np.zeros((80, HG, 128), f32)
    for gp in range(HG):
        for ii in range(4):
            hpatV[5 * gp + ii, gp, 32 * ii:32 * (ii + 1)] = h1[0] * ah   # aH diag
            hpatV[5 * gp + 4, gp, 32 * ii:32 * (ii + 1)] = h1[2] * ah    # wk
    wq = (h1[1] * ah)[None, :].astype(f32)
    b1r = (h1b * ah)[None, :].astype(f32)

    def ccdr(sgn):
        cc = np.zeros((128, HG // 2 * NH, 2, 128), f32)
        for pr in range(HG // 2 * NH):
            for t in range(2):
                gg = 2 * pr + t                # global group -> i rows 4g..4g+3
                for ii in range(4):
                    for c in range(32):
                        cc[32 * ii + c, pr, t, 4 * gg + ii] = sgn[c]
        return cc.astype(NP8)

    ccgDR = ccdr(sg)
    cchDR = ccdr(sh)

    permV = np.zeros((HW, 80), f32)
    for i in range(HW):
        permV[i, 5 * (i // 4) + (i % 4)] = 1.0
    insV = np.zeros((1, 80), f32)
    insV[0, 4::5] = 1.0

    sens = np.asarray(inp["sensor_coords"], f32)[None, :]
    sensr = np.tile(sens, (HW, 1))

    f1wt = np.ascontiguousarray(
        np.asarray(inp["ffn1_w"], f32).reshape(2, 128, 8, 128).transpose(1, 0, 2, 3)).astype(f16)
    f1b = np.ascontiguousarray(np.asarray(inp["ffn1_b"], f32).reshape(8, 128).T)
    f2wt = np.ascontiguousarray(
        np.asarray(inp["ffn2_w"], f32).reshape(8, 128, 256).transpose(1, 0, 2)).astype(f16)
    f2br = np.tile(np.asarray(inp["ffn2_b"], f32)[None, :], (128, 1))
    l1g = np.tile(np.asarray(inp["ln1_g"], f32)[None, :], (128, 1))
    l1b = np.tile(np.asarray(inp["ln1_b"], f32)[None, :], (128, 1))
    l2g = np.tile(np.asarray(inp["ln2_g"], f32)[None, :], (128, 1))
    l2b = np.tile(np.asarray(inp["ln2_b"], f32)[None, :], (128, 1))

    keyT3 = np.ascontiguousarray(key.T.reshape(2, 128, 1024).transpose(1, 0, 2)).astype(f16)
    key3 = np.ascontiguousarray(key.reshape(8, 128, 256).transpose(1, 0, 2)).astype(f16)

    shared = {
        "qx8": qx8, "kx8": kx8,
        "keyT3": keyT3, "key3": key3,
        "g1ab": g1ab, "g1b4": g1b4.astype(f16),
        "hpatV": hpatV.astype(f16), "wq_h": wq, "b1_h": b1r,
        "g2b": np.full((128, 1), g2b, f32), "h2b": np.full((128, 1), h2b, f32),
        "ccgDR": ccgDR, "cchDR": cchDR,
        "permV": permV.astype(f16), "insV": insV.astype(f16),
        "ones16": np.ones((1, 128), f16), "ones32": np.ones((1, 64), f32),
        "i64f": np.eye(64, dtype=f32), "i128f": np.eye(128, dtype=f32),
        "sens": sens, "sensr": sensr,
        "f1w": f1wt, "f1b": f1b, "f2w": f2wt, "f2br": f2br,
        "l1g": l1g, "l1b": l1b, "l2g": l2g, "l2b": l2b,
        "epsc": np.full((128, 1), 1e-6, f32),
    }

    in_maps = []
    for m in range(NCORES):
        sl = slice(m * KSL, (m + 1) * KSL)
        qa = (q_asn[:, sl] * ASCALE).astype(NP8)
        ka = (k_asn[:, sl] * ASCALE).astype(NP8)
        # (N,128) -> (128 pix, NH, NCH, HW)
        qa8 = np.ascontiguousarray(
            qa.reshape(NCH, 128, NH, HW).transpose(1, 2, 0, 3))
        ka8 = np.ascontiguousarray(
            ka.reshape(NCH, 128, 128).transpose(1, 0, 2))
        qT3 = np.ascontiguousarray(
            query[sl].T.reshape(2, 128, 128).transpose(1, 0, 2)).astype(f16)
        im = dict(shared)
        im.update({
            "qa8": qa8, "ka8": ka8, "qT3": qT3,
            "q_sbh": np.ascontiguousarray(query[sl]),
        })
        in_maps.append(im)
    return in_maps


def kernel(**inputs) -> np.ndarray:
    if "nc" not in _cache:
        _cache["nc"] = build_program()
    nc = _cache["nc"]
    in_maps = prep_inputs(inputs)
    res = run_bass_kernel_spmd(nc, in_maps, list(range(NCORES)))
    return np.concatenate([res.results[m]["out"] for m in range(NCORES)], axis=0)


# revision 6
# speedup vs baseline: 1.6238x; 1.1961x over previous
"""Trainium2 Bass kernel for nn_CustomAttentionLayer (sparse_attention).

Strategy (8 NeuronCores, SPMD single launch), v3:
 - Shard the K=1024 query-cluster axis: core m owns rows [128m, 128m+128).
 - Phase 1 (DMA-bound): assignments host-cast to fp8e4 (scaled x256),
   streamed through the PE with the small fp8 feature matrix
   X = [one_hot(iso) | ones | coords] as the moving operand (16 cycles per
   128-pixel chunk). SP queue order: kx, ka, qx, qa, then heavy consts so
   the k-side finishes (and the AllGather starts) as early as possible.
 - Phase 2: R = G*H with (4i x 32c) partition packing, q-side split in two
   64-row halves for overlap. H hidden built on the PE from an interleaved
   V tile [nkk x4 | n_kps] per group (zero-padded (80,128) lhsT); relu off
   PSUM on ACT/DVE with per-partition T14 bias -> fp8, contracted in
   group-pairs with DoubleRow fp8 matmuls. G hidden relu(B4 + AT4 col) in
   fp16 on DVE (4x mode) / Pool, contracted per group in fp16.
   4 hidden-activation slots + 4 rotating half-width H-build psum banks
   decouple the PE / relu / contract pipeline stages.
 - Tail: logits*R, softmax (DMA-transpose), attention, FFN, layernorms.
"""
import numpy as np
import ml_dtypes

import concourse.bass as bass
import concourse.mybir as mybir
import concourse.tile as tile
from concourse.bass_utils import run_bass_kernel_spmd

F32 = mybir.dt.float32
F16 = mybir.dt.float16
F8 = mybir.dt.float8e4
AF = mybir.ActivationFunctionType
OP = mybir.AluOpType
DR = mybir.MatmulPerfMode.DoubleRow
NP8 = ml_dtypes.float8_e4m3

NCORES = 8
K, D, N, NISO = 1024, 256, 65536, 12
KSL = K // NCORES           # 128 rows per core
NCH = N // 128              # 512 contraction chunks
GRP = 64                    # chunks per DMA group
NGRP = NCH // GRP           # 8
XC = 16                     # X feature cols: [oh(12) | ones | cx | cy | 0]
ASCALE = 256.0              # host-side scale on assignments before fp8 cast
NH = 2                      # q-side halves
HW = KSL // NH              # 64 i-rows per half
HG = 16                     # phase-2 groups per half

_cache = {}


# ---------------------------------------------------------------------------
# walrus in this container rejects >1 sync wait per instruction; split extras
# onto single-wait NOPs on the same engine right before the instruction.
def _split_multiwaits(nc):
    ctr = 0
    for f in nc.m.functions:
        for bb in f.blocks:
            for inst in list(bb.instructions):
                si = inst.sync_info
                if si is None:
                    continue
                waits = list(si.on_wait)
                if len(waits) <= 1:
                    continue
                si.on_wait = [waits[-1]]
                pos = None
                for j, cur in enumerate(bb.instructions):
                    if cur.name == inst.name:
                        pos = j
                        break
                assert pos is not None
                for k2, w in enumerate(waits[:-1]):
                    nop = mybir.InstNoOp(
                        name=f"wsplit-{ctr}",
                        sync_info=mybir.SyncInfo(on_wait=[w], on_update=[]),
                        engine=inst.engine,
                        bass_nofuse=True,
                    )
                    ctr += 1
                    nc.register_instruction(nop)
                    bb.instructions.insert(pos + k2, nop)
    return ctr


def build_program():
    nc = bass.Bass()

    def din(name, shape, dt=F32):
        return nc.dram_tensor(name, list(shape), dt, kind="ExternalInput")

    # big streams
    ka8_h = din("ka8", (128, NCH, 128), F8)
    qa8_h = din("qa8", (128, NH, NCH, HW), F8)
    kx8_h = din("kx8", (128, NCH, XC), F8)
    qx8_h = din("qx8", (128, NCH, XC), F8)
    # heavy consts (loaded late on the SP queue)
    qT3_h = din("qT3", (128, 2, 128), F16)
    keyT3_h = din("keyT3", (128, 2, 1024), F16)
    key3_h = din("key3", (128, 8, 256), F16)
    q_sb_h = din("q_sbh", (128, 256))
    f1w_h = din("f1w", (128, 2, 8, 128), F16)
    f1b_h = din("f1b", (128, 8))
    f2w_h = din("f2w", (128, 8, 256), F16)
    f2b_h = din("f2br", (128, 256))
    l1g_h = din("l1g", (128, 256))
    l1b_h = din("l1b", (128, 256))
    l2g_h = din("l2g", (128, 256))
    l2b_h = din("l2b", (128, 256))
    cchDR_h = din("cchDR", (128, HG, 2, 128), F8)    # H contract DR patterns
    ccg16_h = din("ccg16", (128, 2 * HG, 128), F16)  # G contract per group
    # small consts (scalar queue, early)
    eps_h = din("epsc", (128, 1))
    g1ab_h = din("g1ab", (13, 32))
    g1b4_h = din("g1b4", (12, 128), F16)
    hpatV_h = din("hpatV", (80, HG, 128), F16)
    wq_h = din("wq_h", (1, 32))
    b1_h = din("b1_h", (1, 32))
    g2b_h = din("g2b", (128, 1))
    h2b_h = din("h2b", (128, 1))
    permV_h = din("permV", (HW, 80), F16)
    insV_h = din("insV", (1, 80), F16)
    ones16_h = din("ones16", (1, 128), F16)
    ones32_h = din("ones32", (1, 64))
    i64f_h = din("i64f", (64, 64))
    i128f_h = din("i128f", (128, 128))
    sens_h = din("sens", (1, 2))
    sensr_h = din("sensr", (64, 2))

    out_d = nc.dram_tensor("out", [128, 256], F32, kind="ExternalOutput")
    kside_d = nc.dram_tensor("kside", [16, 128], F32)
    kall_d = nc.dram_tensor("kall", [128, 128], F32, addr_space="Shared")

    with tile.TileContext(nc) as tc:
        with (
            tc.tile_pool(name="consts", bufs=1) as cp,
            tc.tile_pool(name="astream", bufs=3) as ap,
            tc.tile_pool(name="sb", bufs=1) as sp,
            tc.tile_pool(name="ppP", bufs=1, space="PSUM") as ppP,
            tc.tile_pool(name="ppH", bufs=4, space="PSUM") as ppH,
        ):
            def cload(dram, eng, tag=None):
                t = cp.tile(list(dram.shape), dram.dtype, tag=tag or dram.name)
                eng.dma_start(out=t[:], in_=dram[:])
                return t

            sc = nc.scalar
            # small consts first, on the scalar queue
            eps_s = cload(eps_h, sc)
            g1ab_s = cload(g1ab_h, sc)
            g1b4_s = cload(g1b4_h, sc)
            hpatV_s = cload(hpatV_h, sc)
            wq_s = cload(wq_h, sc)
            b1_s = cload(b1_h, sc)
            g2b_s = cload(g2b_h, sc)
            h2b_s = cload(h2b_h, sc)
            permV_s = cload(permV_h, sc)
            insV_s = cload(insV_h, sc)
            ones16_s = cload(ones16_h, sc)
            ones32_s = cload(ones32_h, sc)
            i64f_s = cload(i64f_h, sc)
            i128f_s = cload(i128f_h, sc)
            sens_s = cload(sens_h, sc)
            sensr_s = cload(sensr_h, sc)

            # ---------------- phase 1: k-side reduction ----------------
            kx_s = cload(kx8_h, nc.sync)
            psk = ppP.tile([128, XC], F32, tag="gB")        # shares bank w/ hin
            for g in range(NGRP):
                at = ap.tile([128, GRP, 128], F8, tag="ka")
                nc.sync.dma_start(out=at[:], in_=ka8_h[:, g * GRP:(g + 1) * GRP, :])
                for c in range(GRP):
                    nc.tensor.matmul(
                        psk[:], lhsT=at[:, c, :], rhs=kx_s[:, g * GRP + c, :],
                        start=(g == 0 and c == 0),
                        stop=(g == NGRP - 1 and c == GRP - 1),
                    )
            ksb = sp.tile([128, XC], F32, tag="ksb")
            nc.vector.tensor_copy(ksb[:], psk[:])
            pskT = ppH.tile([XC, 128], F32, tag="psH")
            nc.tensor.transpose(pskT[:], ksb[:], i128f_s[:])
            ksbT = sp.tile([XC, 128], F32, tag="ksbT")
            nc.vector.tensor_copy(ksbT[:], pskT[:])
            nc.scalar.dma_start(out=kside_d[:], in_=ksbT[:])
            nc.gpsimd.collective_compute(
                "AllGather", OP.bypass,
                replica_groups=[list(range(NCORES))],
                ins=[kside_d[:]],
                outs=[kall_d[:]],
            )
            kview = kall_d.rearrange("(g c) k -> c g k", c=16)
            dkpT = sp.tile([12, 1024], F32, tag="dkpT")
            nc.scalar.dma_start(out=dkpT[:].rearrange("c (g k) -> c g k", g=8),
                                in_=kview[0:12, :, :])
            ksum_r = sp.tile([1, 1024], F32, tag="ksum_r")
            nc.scalar.dma_start(out=ksum_r[:].rearrange("c (g k) -> c g k", g=8),
                                in_=kview[12:13, :, :])
            kcxs_r = sp.tile([1, 1024], F32, tag="kcxs_r")
            nc.scalar.dma_start(out=kcxs_r[:].rearrange("c (g k) -> c g k", g=8),
                                in_=kview[13:14, :, :])
            kcys_r = sp.tile([1, 1024], F32, tag="kcys_r")
            nc.scalar.dma_start(out=kcys_r[:].rearrange("c (g k) -> c g k", g=8),
                                in_=kview[14:15, :, :])

            # ---------------- q-side half-0 stream ----------------
            qx_s = cload(qx8_h, nc.sync)
            psq0 = ppH.tile([HW, XC], F32, tag="psH")
            for g in range(NGRP):
                at = ap.tile([128, GRP, HW], F8, tag="qa")
                nc.sync.dma_start(out=at[:], in_=qa8_h[:, 0, g * GRP:(g + 1) * GRP, :])
                for c in range(GRP):
                    nc.tensor.matmul(
                        psq0[:], lhsT=at[:, c, :], rhs=qx_s[:, g * GRP + c, :],
                        start=(g == 0 and c == 0),
                        stop=(g == NGRP - 1 and c == GRP - 1),
                    )

            def q_prep(h, psq):
                """q-side stats for half h -> (AT4h, T14h, nqc2)."""
                qT = sp.tile([HW, XC], F32, tag=f"qT{h}")
                nc.vector.tensor_copy(qT[:], psq[:])
                rq = sp.tile([HW, 1], F32, tag=f"rq{h}")
                nc.vector.tensor_scalar_add(rq[:], qT[:, 12:13], ASCALE * 1e-6)
                nc.vector.reciprocal(rq[:], rq[:])
                qn = sp.tile([HW, 13], F32, tag=f"qn{h}")
                nc.vector.tensor_scalar_mul(qn[:], qT[:, 0:13], rq[:])
                qc2 = sp.tile([HW, 2], F32, tag=f"qc2{h}")
                nc.vector.tensor_scalar_mul(qc2[:], qT[:, 13:15], rq[:])
                nqc2 = sp.tile([HW, 2], F32, tag=f"nqc2{h}")
                nc.vector.tensor_scalar_mul(nqc2[:], qc2[:], -1.0)
                d2 = sp.tile([HW, 2], F32, tag=f"d2{h}")
                nc.vector.tensor_sub(d2[:], qc2[:], sensr_s[:])
                nc.vector.tensor_mul(d2[:], d2[:], d2[:])
                nks_c = sp.tile([HW, 1], F32, tag=f"nksc{h}")
                nc.vector.tensor_reduce(nks_c[:], d2[:], mybir.AxisListType.X, OP.add)
                nc.scalar.activation(nks_c[:], nks_c[:], AF.Sqrt)
                pqnT = ppH.tile([13, HW], F32, tag="psH")
                nc.tensor.transpose(pqnT[:], qn[:], i64f_s[:])
                qsT = sp.tile([13, HW], F32, tag=f"qsT{h}")
                nc.vector.tensor_copy(qsT[:], pqnT[:])
                pnksT = ppH.tile([1, HW], F32, tag="psH")
                nc.tensor.transpose(pnksT[:], nks_c[:], i64f_s[:])
                nks_r = sp.tile([1, HW], F32, tag=f"nksr{h}")
                nc.vector.tensor_copy(nks_r[:], pnksT[:])
                psA = ppH.tile([128, HG], F32, tag="psH")
                for ii in range(4):
                    nc.tensor.matmul(psA[32 * ii:32 * (ii + 1), :], lhsT=g1ab_s[:],
                                     rhs=qsT[:, ii::4], start=True, stop=True,
                                     tile_position=(0, 32 * ii))
                AT4h = sp.tile([128, HG], F32, tag=f"AT4{h}")
                nc.vector.tensor_copy(AT4h[:], psA[:])
                psT1 = ppH.tile([128, HG], F32, tag="psH")
                for ii in range(4):
                    nc.tensor.matmul(psT1[32 * ii:32 * (ii + 1), :], lhsT=wq_s[:],
                                     rhs=nks_r[0:1, ii::4], start=True, stop=False,
                                     tile_position=(0, 32 * ii))
                    nc.tensor.matmul(psT1[32 * ii:32 * (ii + 1), :], lhsT=b1_s[:],
                                     rhs=ones32_s[0:1, ii::4], start=False, stop=True,
                                     tile_position=(0, 32 * ii))
                T14h = sp.tile([128, HG], F32, tag=f"T14{h}")
                nc.vector.tensor_copy(T14h[:], psT1[:])
                return AT4h, T14h, nqc2

            AT40, T140, nqc20 = q_prep(0, psq0)

            # ---------------- k-side prep (after collective) ----------------
            rk_r = sp.tile([1, 1024], F32, tag="rk_r")
            nc.vector.tensor_scalar_add(rk_r[:], ksum_r[:], ASCALE * 1e-6)
            nc.vector.reciprocal(rk_r[:], rk_r[:])
            kcx_r = sp.tile([1, 1024], F32, tag="kcx_r")
            nc.vector.tensor_mul(kcx_r[:], kcxs_r[:], rk_r[:])
            kcy_r = sp.tile([1, 1024], F32, tag="kcy_r")
            nc.vector.tensor_mul(kcy_r[:], kcys_r[:], rk_r[:])
            kcx16 = sp.tile([1, 1024], F16, tag="kcx16")
            nc.vector.tensor_copy(kcx16[:], kcx_r[:])
            kcy16 = sp.tile([1, 1024], F16, tag="kcy16")
            nc.vector.tensor_copy(kcy16[:], kcy_r[:])
            s1 = sp.tile([1, 1024], F32, tag="s1")
            s2 = sp.tile([1, 1024], F32, tag="s2")
            nc.vector.tensor_scalar_sub(s1[:], kcx_r[:], sens_s[0:1, 0:1])
            nc.vector.tensor_scalar_sub(s2[:], kcy_r[:], sens_s[0:1, 1:2])
            nc.vector.tensor_mul(s1[:], s1[:], s1[:])
            nc.vector.tensor_mul(s2[:], s2[:], s2[:])
            nc.vector.tensor_add(s1[:], s1[:], s2[:])
            nkps16 = sp.tile([1, 1024], F16, tag="nkps16")
            nc.scalar.activation(nkps16[:], s1[:], AF.Sqrt)
            dkpn = sp.tile([12, 1024], F16, tag="dkpn")
            B4 = sp.tile([128, 1024], F16, tag="B4")
            for b in range(2):
                sl = slice(512 * b, 512 * (b + 1))
                rkb = ppH.tile([12, 512], F32, tag="psH")
                nc.tensor.matmul(rkb[:], lhsT=ones32_s[0:1, 0:12], rhs=rk_r[0:1, sl],
                                 start=True, stop=True)
                nc.vector.tensor_mul(dkpn[:, sl], dkpT[:, sl], rkb[:])
                psB = ppH.tile([128, 512], F32, tag="psH")
                nc.tensor.matmul(psB[:], lhsT=g1b4_s[:], rhs=dkpn[:, sl],
                                 start=True, stop=True)
                nc.scalar.activation(B4[:, sl], psB[:], AF.Copy)

            # ---------------- q-side half-1 stream ----------------
            psq1 = ppP.tile([HW, XC], F32, tag="gA")        # shares bank w/ gin
            for g in range(NGRP):
                at = ap.tile([128, GRP, HW], F8, tag="qa")
                nc.sync.dma_start(out=at[:], in_=qa8_h[:, 1, g * GRP:(g + 1) * GRP, :])
                for c in range(GRP):
                    nc.tensor.matmul(
                        psq1[:], lhsT=at[:, c, :], rhs=qx_s[:, g * GRP + c, :],
                        start=(g == 0 and c == 0),
                        stop=(g == NGRP - 1 and c == GRP - 1),
                    )

            # heavy consts stream behind the last qa group on SP
            cch_s = cload(cchDR_h, nc.sync)
            ccg_s = cload(ccg16_h, nc.sync)
            qT3_s = cload(qT3_h, nc.sync)
            keyT3_s = cload(keyT3_h, nc.sync)
            key3_s = cload(key3_h, nc.sync)
            f1w_s = cload(f1w_h, nc.sync)
            f2w_s = cload(f2w_h, nc.sync)
            qsbh_s = cload(q_sb_h, nc.sync)
            f1b_s = cload(f1b_h, nc.sync)
            f2b_s = cload(f2b_h, nc.sync)
            l1g_s = cload(l1g_h, nc.sync)
            l1b_s = cload(l1b_h, nc.sync)
            l2g_s = cload(l2g_h, nc.sync)
            l2b_s = cload(l2b_h, nc.sync)

            def nkk_v(h, nqc2):
                """nkk + interleaved V80 for half h."""
                dx2 = sp.tile([HW, 1024], F16, tag=f"dx2{h}")
                dy2 = sp.tile([HW, 1024], F16, tag=f"dy2{h}")
                for b in range(2):
                    sl = slice(512 * b, 512 * (b + 1))
                    pK = ppH.tile([HW, 512], F32, tag="psH")
                    nc.tensor.matmul(pK[:], lhsT=ones16_s[0:1, 0:HW], rhs=kcx16[0:1, sl],
                                     start=True, stop=True)
                    nc.scalar.activation(dx2[:, sl], pK[:], AF.Square,
                                         bias=nqc2[:, 0:1])
                    pK2 = ppH.tile([HW, 512], F32, tag="psH")
                    nc.tensor.matmul(pK2[:], lhsT=ones16_s[0:1, 0:HW], rhs=kcy16[0:1, sl],
                                     start=True, stop=True)
                    nc.scalar.activation(dy2[:, sl], pK2[:], AF.Square,
                                         bias=nqc2[:, 1:2])
                nc.vector.tensor_add(dx2[:], dx2[:], dy2[:])
                nkk = sp.tile([HW, 1024], F16, tag=f"nkk{h}")
                nc.scalar.activation(nkk[:], dx2[:], AF.Sqrt)
                V80 = sp.tile([80, 1024], F16, tag=f"V80{h}")
                for b in range(2):
                    sl = slice(512 * b, 512 * (b + 1))
                    pV = ppH.tile([80, 512], F32, tag="psH")
                    nc.tensor.matmul(pV[:], lhsT=permV_s[:], rhs=nkk[:, sl],
                                     start=True, stop=False)
                    nc.tensor.matmul(pV[:], lhsT=insV_s[:], rhs=nkps16[0:1, sl],
                                     start=False, stop=True)
                    nc.scalar.activation(V80[:, sl], pV[:], AF.Copy)
                return V80

            V800 = nkk_v(0, nqc20)

            # ---------------- phase 2 ----------------
            gin = ppP.tile([128, 1024], F32, tag="gA")
            hin = ppP.tile([128, 1024], F32, tag="gB")
            ph2 = sp.tile([128, 4, 1024], F8, tag="ph2")
            pg2 = sp.tile([128, 4, 1024], F16, tag="pg2")

            A, Dv, P = nc.scalar, nc.vector, nc.gpsimd
            hlo_eng = [A, Dv] * 8
            hhi_eng = [Dv, A] * 8
            g_eng = ([Dv, Dv, P] * 6)[:HG]

            def _relu(eng, dst, src, bcol):
                if eng is A:
                    nc.scalar.activation(dst, src, AF.Relu, bias=bcol)
                else:
                    eng.tensor_scalar(dst, src, bcol, 0.0, OP.add, OP.max)

            def phase2_half(h, AT4h, T14h, V80):
                for gp in range(HG):
                    g = HG * h + gp
                    s = g % 4
                    psa = ppH.tile([128, 512], F32, tag="psH")
                    nc.tensor.matmul(psa[:], lhsT=hpatV_s[:, gp, :],
                                     rhs=V80[:, 0:512], start=True, stop=True)
                    psb = ppH.tile([128, 512], F32, tag="psH")
                    nc.tensor.matmul(psb[:], lhsT=hpatV_s[:, gp, :],
                                     rhs=V80[:, 512:1024], start=True, stop=True)
                    _relu(hlo_eng[gp], ph2[:, s, 0:512], psa[:], T14h[:, gp:gp + 1])
                    _relu(hhi_eng[gp], ph2[:, s, 512:1024], psb[:], T14h[:, gp:gp + 1])
                    _relu(g_eng[gp], pg2[:, s, :], B4[:], AT4h[:, gp:gp + 1])
                    for b in range(2):
                        sl = slice(512 * b, 512 * (b + 1))
                        nc.tensor.matmul(gin[:, sl], lhsT=ccg_s[:, g, :],
                                         rhs=pg2[:, s, sl],
                                         start=(g == 0), stop=(g == 2 * HG - 1))
                    if s % 2 == 1:
                        pr = g // 2
                        t0 = 2 * (pr % 2)
                        for b in range(2):
                            sl = slice(512 * b, 512 * (b + 1))
                            nc.tensor.matmul(hin[:, sl], lhsT=cch_s[:, pr, :, :],
                                             rhs=ph2[:, t0:t0 + 2, sl], perf_mode=DR,
                                             start=(pr == 0), stop=(pr == HG - 1))

            phase2_half(0, AT40, T140, V800)

            # logits (needs late consts; runs in the h0 drain window)
            lraw = sp.tile([128, 1024], F16, tag="lraw")
            for b in range(2):
                psl = ppH.tile([128, 512], F32, tag="psH")
                for c in range(2):
                    nc.tensor.matmul(psl[:], lhsT=qT3_s[:, c, :],
                                     rhs=keyT3_s[:, c, 512 * b:512 * (b + 1)],
                                     start=(c == 0), stop=(c == 1))
                nc.scalar.activation(lraw[:, 512 * b:512 * (b + 1)], psl[:],
                                     AF.Copy, scale=1.0 / 16.0)

            AT41, T141, nqc21 = q_prep(1, psq1)
            V801 = nkk_v(1, nqc21)
            phase2_half(1, AT41, T141, V801)

            # ---------------- tail: R, softmax, attention, FFN, LN ----------
            rg = sp.tile([128, 1024], F16, tag="rg")
            nc.scalar.activation(rg[:], gin[:], AF.Relu, bias=g2b_s[:])
            rh = sp.tile([128, 1024], F16, tag="rh")
            nc.vector.tensor_scalar(rh[:], hin[:], h2b_s[:], 0.0, OP.add, OP.max)
            lg = sp.tile([128, 1024], F16, tag="lg")
            nc.vector.tensor_mul(lg[:], rg[:], rh[:])
            nc.vector.tensor_mul(lg[:], lg[:], lraw[:])
            mx = sp.tile([128, 1], F32, tag="mx")
            nc.vector.tensor_reduce(mx[:], lg[:], mybir.AxisListType.X, OP.max)
            nmx = sp.tile([128, 1], F32, tag="nmx")
            nc.vector.tensor_scalar_mul(nmx[:], mx[:], -1.0)
            pexp = sp.tile([128, 1024], F16, tag="pexp")
            sume = sp.tile([128, 1], F32, tag="sume")
            nc.scalar.activation(pexp[:], lg[:], AF.Exp, bias=nmx[:], accum_out=sume[:])
            rsum = sp.tile([128, 1], F32, tag="rsum")
            nc.vector.reciprocal(rsum[:], sume[:])
            pT = sp.tile([128, 8, 128], F16, tag="pT")
            nc.scalar.dma_start_transpose(pT[:], pexp[:])
            attn = ppH.tile([128, 256], F32, tag="psH")
            for t in range(8):
                nc.tensor.matmul(attn[:], lhsT=pT[:, t, :], rhs=key3_s[:, t, :],
                                 start=(t == 0), stop=(t == 7))
            xpre = sp.tile([128, 256], F32, tag="xpre")
            nc.vector.scalar_tensor_tensor(xpre[:], attn[:], rsum[:], qsbh_s[:],
                                           op0=OP.mult, op1=OP.add)

            def layer_norm(src, gt, bt, tag):
                m = sp.tile([128, 1], F32, tag=f"m_{tag}")
                nc.vector.tensor_reduce(m[:], src[:], mybir.AxisListType.X, OP.add)
                nc.vector.tensor_scalar_mul(m[:], m[:], -1.0 / 256.0)
                xc = sp.tile([128, 256], F32, tag=f"xc_{tag}")
                nc.vector.tensor_scalar_add(xc[:], src[:], m[:])
                var = sp.tile([128, 1], F32, tag=f"v_{tag}")
                xc2 = sp.tile([128, 256], F32, tag=f"xc2_{tag}")
                nc.vector.scalar_tensor_tensor(xc2[:], xc[:], 1.0, xc[:],
                                               op0=OP.mult, op1=OP.mult, accum_out=var[:])
                lnv = sp.tile([128, 1], F32, tag=f"lv_{tag}")
                nc.scalar.activation(lnv[:], var[:], AF.Ln, scale=1.0 / 256.0, bias=eps_s[:])
                rstd = sp.tile([128, 1], F32, tag=f"rs_{tag}")
                nc.scalar.activation(rstd[:], lnv[:], AF.Exp, scale=-0.5)
                y = sp.tile([128, 256], F32, tag=f"y_{tag}")
                nc.vector.scalar_tensor_tensor(y[:], xc[:], rstd[:], gt[:],
                                               op0=OP.mult, op1=OP.mult)
                nc.vector.tensor_add(y[:], y[:], bt[:])
                return y

            x1 = layer_norm(xpre, l1g_s, l1b_s, "ln1")
            x1h = sp.tile([128, 256], F16, tag="x1h")
            nc.vector.tensor_copy(x1h[:], x1[:])
            xT = sp.tile([128, 2, 128], F16, tag="xT")
            nc.scalar.dma_start_transpose(xT[:], x1h[:])
            hT = sp.tile([128, 8, 128], F16, tag="hT")
            for t in range(8):
                psh = ppH.tile([128, 128], F32, tag="psH")
                for c in range(2):
                    nc.tensor.matmul(psh[:], lhsT=f1w_s[:, c, t, :], rhs=xT[:, c, :],
                                     start=(c == 0), stop=(c == 1))
                if t % 2 == 0:
                    nc.scalar.activation(hT[:, t, :], psh[:], AF.Relu, bias=f1b_s[:, t:t + 1])
                else:
                    nc.vector.tensor_scalar(hT[:, t, :], psh[:], f1b_s[:, t:t + 1],
                                            0.0, OP.add, OP.max)
            pso = ppH.tile([128, 256], F32, tag="psH")
            for t in range(8):
                nc.tensor.matmul(pso[:], lhsT=hT[:, t, :], rhs=f2w_s[:, t, :],
                                 start=(t == 0), stop=(t == 7))
            y2 = sp.tile([128, 256], F32, tag="y2")
            nc.vector.tensor_add(y2[:], pso[:], x1[:])
            nc.vector.tensor_add(y2[:], y2[:], f2b_s[:])
            x2 = layer_norm(y2, l2g_s, l2b_s, "ln2")
            nc.scalar.dma_start(out=out_d[:], in_=x2[:])

    _split_multiwaits(nc)
    return nc


# ---------------------------------------------------------------------------
def prep_inputs(inp):
    f32 = np.float32
    f16 = np.float16
    q_asn = np.asarray(inp["q_assignments"], f32)
    k_asn = np.asarray(inp["k_assignments"], f32)
    query = np.asarray(inp["query"], f32)
    key = np.asarray(inp["key_emb"], f32)

    def xfeat(coords, iso):
        oh = np.zeros((N, NISO), f32)
        oh[np.arange(N), np.asarray(iso) - 1] = 1.0
        x = np.concatenate([oh, np.ones((N, 1), f32), np.asarray(coords, f32),
                            np.zeros((N, 1), f32)], axis=1)
        return np.ascontiguousarray(
            x.reshape(NCH, 128, XC).transpose(1, 0, 2)).astype(NP8)

    qx8 = xfeat(inp["q_coords"], inp["q_iso"])
    kx8 = xfeat(inp["k_coords"], inp["k_iso"])

    g1 = np.asarray(inp["g1_w"], f32)
    g1b = np.asarray(inp["g1_b"], f32)
    g2 = np.asarray(inp["g2_w"], f32)[:, 0]
    g2b = float(np.asarray(inp["g2_b"], f32)[0])
    h1 = np.asarray(inp["h1_w"], f32)
    h1b = np.asarray(inp["h1_b"], f32)
    h2 = np.asarray(inp["h2_w"], f32)[:, 0]
    h2b = float(np.asarray(inp["h2_b"], f32)[0])

    ag, sg = np.abs(g2), np.sign(g2).astype(f32)
    ah, sh = np.abs(h2), np.sign(h2).astype(f32)

    g1ab = np.concatenate([g1[:12] * ag[None, :], (g1b * ag)[None, :]], axis=0)
    g1b4 = np.tile(g1[12:] * ag[None, :], (1, 4))

    hpatV = np.zeros((80, HG, 128), f32)
    for gp in range(HG):
        for ii in range(4):
            hpatV[5 * gp + ii, gp, 32 * ii:32 * (ii + 1)] = h1[0] * ah
            hpatV[5 * gp + 4, gp, 32 * ii:32 * (ii + 1)] = h1[2] * ah
    wq = (h1[1] * ah)[None, :].astype(f32)
    b1r = (h1b * ah)[None, :].astype(f32)

    # H contract: DoubleRow pattern per global pair pr (groups 2pr, 2pr+1),
    # writing gin/hin columns 4g..4g+3.
    cchDR = np.zeros((128, HG, 2, 128), f32)
    for pr in range(HG):
        for t in range(2):
            gg = 2 * pr + t
            for ii in range(4):
                for c in range(32):
                    cchDR[32 * ii + c, pr, t, 4 * gg + ii] = sh[c]
    cchDR = cchDR.astype(NP8)

    ccg16 = np.zeros((128, 2 * HG, 128), f32)
    for g in range(2 * HG):
        for ii in range(4):
            for c in range(32):
                ccg16[32 * ii + c, g, 4 * g + ii] = sg[c]
    ccg16 = ccg16.astype(f16)

    permV = np.zeros((HW, 80), f32)
    for i in range(HW):
        permV[i, 5 * (i // 4) + (i % 4)] = 1.0
    insV = np.zeros((1, 80), f32)
    insV[0, 4::5] = 1.0

    sens = np.asarray(inp["sensor_coords"], f32)[None, :]
    sensr = np.tile(sens, (HW, 1))

    f1wt = np.ascontiguousarray(
        np.asarray(inp["ffn1_w"], f32).reshape(2, 128, 8, 128).transpose(1, 0, 2, 3)).astype(f16)
    f1b = np.ascontiguousarray(np.asarray(inp["ffn1_b"], f32).reshape(8, 128).T)
    f2wt = np.ascontiguousarray(
        np.asarray(inp["ffn2_w"], f32).reshape(8, 128, 256).transpose(1, 0, 2)).astype(f16)
    f2br = np.tile(np.asarray(inp["ffn2_b"], f32)[None, :], (128, 1))
    l1g = np.tile(np.asarray(inp["ln1_g"], f32)[None, :], (128, 1))
    l1b = np.tile(np.asarray(inp["ln1_b"], f32)[None, :], (128, 1))
    l2g = np.tile(np.asarray(inp["ln2_g"], f32)[None, :], (128, 1))
    l2b = np.tile(np.asarray(inp["ln2_b"], f32)[None, :], (128, 1))

    keyT3 = np.ascontiguousarray(key.T.reshape(2, 128, 1024).transpose(1, 0, 2)).astype(f16)
    key3 = np.ascontiguousarray(key.reshape(8, 128, 256).transpose(1, 0, 2)).astype(f16)

    shared = {
        "qx8": qx8, "kx8": kx8,
        "keyT3": keyT3, "key3": key3,
        "g1ab": g1ab, "g1b4": g1b4.astype(f16),
        "hpatV": hpatV.astype(f16), "wq_h": wq, "b1_h": b1r,
        "g2b": np.full((128, 1), g2b, f32), "h2b": np.full((128, 1), h2b, f32),
        "cchDR": cchDR, "ccg16": ccg16,
        "permV": permV.astype(f16), "insV": insV.astype(f16),
        "ones16": np.ones((1, 128), f16), "ones32": np.ones((1, 64), f32),
        "i64f": np.eye(64, dtype=f32), "i128f": np.eye(128, dtype=f32),
        "sens": sens, "sensr": sensr,
        "f1w": f1wt, "f1b": f1b, "f2w": f2wt, "f2br": f2br,
        "l1g": l1g, "l1b": l1b, "l2g": l2g, "l2b": l2b,
        "epsc": np.full((128, 1), 1e-6, f32),
    }

    in_maps = []
    for m in range(NCORES):
        sl = slice(m * KSL, (m + 1) * KSL)
        qa = (q_asn[:, sl] * ASCALE).astype(NP8)
        ka = (k_asn[:, sl] * ASCALE).astype(NP8)
        qa8 = np.ascontiguousarray(
            qa.reshape(NCH, 128, NH, HW).transpose(1, 2, 0, 3))
        ka8 = np.ascontiguousarray(
            ka.reshape(NCH, 128, 128).transpose(1, 0, 2))
        qT3 = np.ascontiguousarray(
            query[sl].T.reshape(2, 128, 128).transpose(1, 0, 2)).astype(f16)
        im = dict(shared)
        im.update({
            "qa8": qa8, "ka8": ka8, "qT3": qT3,
            "q_sbh": np.ascontiguousarray(query[sl]),
        })
        in_maps.append(im)
    return in_maps


def kernel(**inputs) -> np.ndarray:
    if "nc" not in _cache:
        _cache["nc"] = build_program()
    nc = _cache["nc"]
    in_maps = prep_inputs(inputs)
    res = run_bass_kernel_spmd(nc, in_maps, list(range(NCORES)))
    return np.concatenate([res.results[m]["out"] for m in range(NCORES)], axis=0)


# revision 8
# speedup vs baseline: 1.7949x; 1.1054x over previous
"""Trainium2 Bass kernel for nn_CustomAttentionLayer (sparse_attention).

Strategy (8 NeuronCores, SPMD single launch), v3:
 - Shard the K=1024 query-cluster axis: core m owns rows [128m, 128m+128).
 - Phase 1 (DMA-bound): assignments host-cast to fp8e4 (scaled x256),
   streamed through the PE with the small fp8 feature matrix
   X = [one_hot(iso) | ones | coords] as the moving operand (16 cycles per
   128-pixel chunk). SP queue order: kx, ka, qx, qa, then heavy consts so
   the k-side finishes (and the AllGather starts) as early as possible.
 - Phase 2: R = G*H with (4i x 32c) partition packing, q-side split in two
   64-row halves for overlap. H hidden built on the PE from an interleaved
   V tile [nkk x4 | n_kps] per group (zero-padded (80,128) lhsT); relu off
   PSUM on ACT/DVE with per-partition T14 bias -> fp8, contracted in
   group-pairs with DoubleRow fp8 matmuls. G hidden relu(B4 + AT4 col) in
   fp16 on DVE (4x mode) / Pool, contracted per group in fp16.
   4 hidden-activation slots + 4 rotating half-width H-build psum banks
   decouple the PE / relu / contract pipeline stages.
 - Tail: logits*R, softmax (DMA-transpose), attention, FFN, layernorms.
"""
import numpy as np
import ml_dtypes

import concourse.bass as bass
import concourse.mybir as mybir
import concourse.tile as tile
from concourse.bass_utils import run_bass_kernel_spmd

F32 = mybir.dt.float32
F16 = mybir.dt.float16
F8 = mybir.dt.float8e4
AF = mybir.ActivationFunctionType
OP = mybir.AluOpType
DR = mybir.MatmulPerfMode.DoubleRow
NP8 = ml_dtypes.float8_e4m3

NCORES = 8
K, D, N, NISO = 1024, 256, 65536, 12
KSL = K // NCORES           # 128 rows per core
NCH = N // 128              # 512 contraction chunks
GRP = 64                    # chunks per DMA group
NGRP = NCH // GRP           # 8
XC = 16                     # X feature cols: [oh(12) | ones | cx | cy | 0]
ASCALE = 256.0              # host-side scale on assignments before fp8 cast
NH = 2                      # q-side halves
HW = KSL // NH              # 64 i-rows per half
HG = 16                     # phase-2 groups per half

_cache = {}


# ---------------------------------------------------------------------------
# walrus in this container rejects >1 sync wait per instruction; split extras
# onto single-wait NOPs on the same engine right before the instruction.
def _split_multiwaits(nc):
    ctr = 0
    for f in nc.m.functions:
        for bb in f.blocks:
            for inst in list(bb.instructions):
                si = inst.sync_info
                if si is None:
                    continue
                waits = list(si.on_wait)
                if len(waits) <= 1:
                    continue
                si.on_wait = [waits[-1]]
                pos = None
                for j, cur in enumerate(bb.instructions):
                    if cur.name == inst.name:
                        pos = j
                        break
                assert pos is not None
                for k2, w in enumerate(waits[:-1]):
                    nop = mybir.InstNoOp(
                        name=f"wsplit-{ctr}",
                        sync_info=mybir.SyncInfo(on_wait=[w], on_update=[]),
                        engine=inst.engine,
                        bass_nofuse=True,
                    )
                    ctr += 1
                    nc.register_instruction(nop)
                    bb.instructions.insert(pos + k2, nop)
    return ctr


def build_program():
    nc = bass.Bass()

    def din(name, shape, dt=F32):
        return nc.dram_tensor(name, list(shape), dt, kind="ExternalInput")

    # big streams
    ka8_h = din("ka8", (128, NCH, 128), F8)
    qa8_h = din("qa8", (128, NH, NCH, HW), F8)
    kx8_h = din("kx8", (128, NCH, XC), F8)
    qx8_h = din("qx8", (128, NCH, XC), F8)
    # heavy consts (loaded late on the SP queue)
    qT3_h = din("qT3", (128, 2, 128), F16)
    keyT3_h = din("keyT3", (128, 2, 1024), F16)
    key3_h = din("key3", (128, 8, 256), F16)
    q_sb_h = din("q_sbh", (128, 256))
    f1w_h = din("f1w", (128, 2, 8, 128), F16)
    f1b_h = din("f1b", (128, 8))
    f2w_h = din("f2w", (128, 8, 256), F16)
    f2b_h = din("f2br", (128, 256))
    l1g_h = din("l1g", (128, 256))
    l1b_h = din("l1b", (128, 256))
    l2g_h = din("l2g", (128, 256))
    l2b_h = din("l2b", (128, 256))
    cchDR_h = din("cchDR", (128, HG, 2, 128), F8)    # H contract DR patterns
    ccg16_h = din("ccg16", (128, 2 * HG, 128), F16)  # G contract per group
    # small consts (scalar queue, early)
    eps_h = din("epsc", (128, 1))
    g1ab_h = din("g1ab", (13, 32))
    g1b4_h = din("g1b4", (12, 128), F16)
    hpatV_h = din("hpatV", (80, HG, 128), F16)
    wq_h = din("wq_h", (1, 32))
    b1_h = din("b1_h", (1, 32))
    g2b_h = din("g2b", (128, 1))
    h2b_h = din("h2b", (128, 1))
    permV_h = din("permV", (HW, 80), F16)
    insV_h = din("insV", (1, 80), F16)
    ones16_h = din("ones16", (1, 128), F16)
    ones32_h = din("ones32", (1, 64))
    i64f_h = din("i64f", (64, 64))
    i128f_h = din("i128f", (128, 128))
    i128h_h = din("i128h", (128, 128), F16)
    sensr128_h = din("sensr128", (128, 2))
    sens_h = din("sens", (1, 2))
    sensr_h = din("sensr", (64, 2))

    out_d = nc.dram_tensor("out", [128, 256], F32, kind="ExternalOutput")
    kside_d = nc.dram_tensor("kside", [16, 128], F16)
    kall_d = nc.dram_tensor("kall", [128, 128], F16, addr_space="Shared")

    with tile.TileContext(nc) as tc:
        with (
            tc.tile_pool(name="consts", bufs=1) as cp,
            tc.tile_pool(name="astream", bufs=3) as ap,
            tc.tile_pool(name="sb", bufs=1) as sp,
            tc.tile_pool(name="ppP", bufs=1, space="PSUM") as ppP,
            tc.tile_pool(name="ppH", bufs=4, space="PSUM") as ppH,
        ):
            def cload(dram, eng, tag=None):
                t = cp.tile(list(dram.shape), dram.dtype, tag=tag or dram.name)
                eng.dma_start(out=t[:], in_=dram[:])
                return t

            sc = nc.gpsimd
            # small consts via SWDGE so HWDGE stays free for the A stream
            eps_s = cload(eps_h, sc)
            g1ab_s = cload(g1ab_h, sc)
            g1b4_s = cload(g1b4_h, sc)
            hpatV_s = cload(hpatV_h, sc)
            wq_s = cload(wq_h, sc)
            b1_s = cload(b1_h, sc)
            g2b_s = cload(g2b_h, sc)
            h2b_s = cload(h2b_h, sc)
            permV_s = cload(permV_h, sc)
            insV_s = cload(insV_h, sc)
            ones16_s = cload(ones16_h, sc)
            ones32_s = cload(ones32_h, sc)
            i64f_s = cload(i64f_h, sc)
            i128f_s = cload(i128f_h, sc)
            i128h_s = cload(i128h_h, sc)
            sensr128_s = cload(sensr128_h, sc)
            sens_s = cload(sens_h, sc)
            sensr_s = cload(sensr_h, sc)

            # ---------------- phase 1: k-side reduction ----------------
            kx_s = cload(kx8_h, nc.sync)
            psk = ppP.tile([128, XC], F32, tag="gB")        # shares bank w/ hin
            for g in range(NGRP):
                at = ap.tile([128, GRP, 128], F8, tag="ka")
                nc.sync.dma_start(out=at[:], in_=ka8_h[:, g * GRP:(g + 1) * GRP, :])
                for c in range(GRP):
                    nc.tensor.matmul(
                        psk[:], lhsT=at[:, c, :], rhs=kx_s[:, g * GRP + c, :],
                        start=(g == 0 and c == 0),
                        stop=(g == NGRP - 1 and c == GRP - 1),
                    )
            # pre-normalize this core's k-slab (tiny per-partition col ops)
            ksb = sp.tile([128, XC], F32, tag="ksb")
            nc.vector.tensor_copy(ksb[:], psk[:])
            rkc = sp.tile([128, 1], F32, tag="rkc")
            nc.vector.tensor_scalar_add(rkc[:], ksb[:, 12:13], ASCALE * 1e-6)
            nc.vector.reciprocal(rkc[:], rkc[:])
            kprep = sp.tile([128, XC], F16, tag="kprep")
            nc.vector.tensor_scalar_mul(kprep[:, 0:12], ksb[:, 0:12], rkc[:])
            kc2 = sp.tile([128, 2], F32, tag="kc2")
            nc.vector.tensor_scalar_mul(kc2[:], ksb[:, 13:15], rkc[:])
            nc.vector.tensor_copy(kprep[:, 13:15], kc2[:])
            dk2 = sp.tile([128, 2], F32, tag="dk2")
            nc.vector.tensor_sub(dk2[:], kc2[:], sensr128_s[:])
            nc.vector.tensor_mul(dk2[:], dk2[:], dk2[:])
            nkp_c = sp.tile([128, 1], F32, tag="nkp_c")
            nc.vector.tensor_reduce(nkp_c[:], dk2[:], mybir.AxisListType.X, OP.add)
            nc.scalar.activation(kprep[:, 12:13], nkp_c[:], AF.Sqrt)
            nc.vector.tensor_copy(kprep[:, 15:16], kc2[:, 0:1])  # pad
            pskT = ppH.tile([XC, 128], F16, tag="psH")
            nc.tensor.transpose(pskT[:], kprep[:], i128h_s[:])
            ksbT = sp.tile([XC, 128], F16, tag="ksbT")
            nc.vector.tensor_copy(ksbT[:], pskT[:])
            nc.sync.dma_start(out=kside_d[:], in_=ksbT[:])
            nc.gpsimd.collective_compute(
                "AllGather", OP.bypass,
                replica_groups=[list(range(NCORES))],
                ins=[kside_d[:]],
                outs=[kall_d[:]],
            )
            kview = kall_d.rearrange("(g c) k -> c g k", c=16)
            dkpn = sp.tile([12, 1024], F16, tag="dkpn")
            nc.scalar.dma_start(out=dkpn[:].rearrange("c (g k) -> c g k", g=8),
                                in_=kview[0:12, :, :])
            nkps16 = sp.tile([1, 1024], F16, tag="nkps16")
            nc.sync.dma_start(out=nkps16[:].rearrange("c (g k) -> c g k", g=8),
                              in_=kview[12:13, :, :])
            kcx16 = sp.tile([1, 1024], F16, tag="kcx16")
            nc.scalar.dma_start(out=kcx16[:].rearrange("c (g k) -> c g k", g=8),
                                in_=kview[13:14, :, :])
            kcy16 = sp.tile([1, 1024], F16, tag="kcy16")
            nc.sync.dma_start(out=kcy16[:].rearrange("c (g k) -> c g k", g=8),
                              in_=kview[14:15, :, :])

            # ---------------- q-side half-0 stream ----------------
            qx_s = cload(qx8_h, nc.sync)
            psq0 = ppH.tile([HW, XC], F32, tag="psH")
            for g in range(NGRP):
                at = ap.tile([128, GRP, HW], F8, tag="qa")
                nc.sync.dma_start(out=at[:], in_=qa8_h[:, 0, g * GRP:(g + 1) * GRP, :])
                for c in range(GRP):
                    nc.tensor.matmul(
                        psq0[:], lhsT=at[:, c, :], rhs=qx_s[:, g * GRP + c, :],
                        start=(g == 0 and c == 0),
                        stop=(g == NGRP - 1 and c == GRP - 1),
                    )

            def q_prep(h, psq):
                """q-side stats for half h -> (AT4h, T14h, nqc2)."""
                qT = sp.tile([HW, XC], F32, tag=f"qT{h}")
                nc.vector.tensor_copy(qT[:], psq[:])
                rq = sp.tile([HW, 1], F32, tag=f"rq{h}")
                nc.vector.tensor_scalar_add(rq[:], qT[:, 12:13], ASCALE * 1e-6)
                nc.vector.reciprocal(rq[:], rq[:])
                qn = sp.tile([HW, 13], F32, tag=f"qn{h}")
                nc.vector.tensor_scalar_mul(qn[:], qT[:, 0:13], rq[:])
                qc2 = sp.tile([HW, 2], F32, tag=f"qc2{h}")
                nc.vector.tensor_scalar_mul(qc2[:], qT[:, 13:15], rq[:])
                nqc2 = sp.tile([HW, 2], F32, tag=f"nqc2{h}")
                nc.vector.tensor_scalar_mul(nqc2[:], qc2[:], -1.0)
                d2 = sp.tile([HW, 2], F32, tag=f"d2{h}")
                nc.vector.tensor_sub(d2[:], qc2[:], sensr_s[:])
                nc.vector.tensor_mul(d2[:], d2[:], d2[:])
                nks_c = sp.tile([HW, 1], F32, tag=f"nksc{h}")
                nc.vector.tensor_reduce(nks_c[:], d2[:], mybir.AxisListType.X, OP.add)
                nc.scalar.activation(nks_c[:], nks_c[:], AF.Sqrt)
                pqnT = ppH.tile([13, HW], F32, tag="psH")
                nc.tensor.transpose(pqnT[:], qn[:], i64f_s[:])
                qsT = sp.tile([13, HW], F32, tag=f"qsT{h}")
                nc.vector.tensor_copy(qsT[:], pqnT[:])
                pnksT = ppH.tile([1, HW], F32, tag="psH")
                nc.tensor.transpose(pnksT[:], nks_c[:], i64f_s[:])
                nks_r = sp.tile([1, HW], F32, tag=f"nksr{h}")
                nc.vector.tensor_copy(nks_r[:], pnksT[:])
                psA = ppH.tile([128, HG], F32, tag="psH")
                for ii in range(4):
                    nc.tensor.matmul(psA[32 * ii:32 * (ii + 1), :], lhsT=g1ab_s[:],
                                     rhs=qsT[:, ii::4], start=True, stop=True,
                                     tile_position=(0, 32 * ii))
                AT4h = sp.tile([128, HG], F32, tag=f"AT4{h}")
                nc.vector.tensor_copy(AT4h[:], psA[:])
                psT1 = ppH.tile([128, HG], F32, tag="psH")
                for ii in range(4):
                    nc.tensor.matmul(psT1[32 * ii:32 * (ii + 1), :], lhsT=wq_s[:],
                                     rhs=nks_r[0:1, ii::4], start=True, stop=False,
                                     tile_position=(0, 32 * ii))
                    nc.tensor.matmul(psT1[32 * ii:32 * (ii + 1), :], lhsT=b1_s[:],
                                     rhs=ones32_s[0:1, ii::4], start=False, stop=True,
                                     tile_position=(0, 32 * ii))
                T14h = sp.tile([128, HG], F32, tag=f"T14{h}")
                nc.vector.tensor_copy(T14h[:], psT1[:])
                return AT4h, T14h, nqc2

            AT40, T140, nqc20 = q_prep(0, psq0)

            # ---------------- k-side prep (after collective): just B4 -------
            B4 = sp.tile([128, 1024], F16, tag="B4")
            for b in range(2):
                sl = slice(512 * b, 512 * (b + 1))
                psB = ppH.tile([128, 512], F32, tag="psH")
                nc.tensor.matmul(psB[:], lhsT=g1b4_s[:], rhs=dkpn[0:12, sl],
                                 start=True, stop=True)
                nc.scalar.activation(B4[:, sl], psB[:], AF.Copy)

            # ---------------- q-side half-1 stream ----------------
            psq1 = ppP.tile([HW, XC], F32, tag="gA")        # shares bank w/ gin
            for g in range(NGRP):
                at = ap.tile([128, GRP, HW], F8, tag="qa")
                nc.sync.dma_start(out=at[:], in_=qa8_h[:, 1, g * GRP:(g + 1) * GRP, :])
                for c in range(GRP):
                    nc.tensor.matmul(
                        psq1[:], lhsT=at[:, c, :], rhs=qx_s[:, g * GRP + c, :],
                        start=(g == 0 and c == 0),
                        stop=(g == NGRP - 1 and c == GRP - 1),
                    )

            # heavy consts stream behind the last qa group on SP
            cch_s = cload(cchDR_h, nc.sync)
            ccg_s = cload(ccg16_h, nc.sync)
            qT3_s = cload(qT3_h, nc.sync)
            keyT3_s = cload(keyT3_h, nc.sync)
            key3_s = cload(key3_h, nc.sync)
            f1w_s = cload(f1w_h, nc.sync)
            f2w_s = cload(f2w_h, nc.sync)
            qsbh_s = cload(q_sb_h, nc.sync)
            f1b_s = cload(f1b_h, nc.sync)
            f2b_s = cload(f2b_h, nc.sync)
            l1g_s = cload(l1g_h, nc.sync)
            l1b_s = cload(l1b_h, nc.sync)
            l2g_s = cload(l2g_h, nc.sync)
            l2b_s = cload(l2b_h, nc.sync)

            def nkk_v(h, nqc2):
                """nkk + interleaved V80 for half h."""
                dx2 = sp.tile([HW, 1024], F16, tag=f"dx2{h}")
                dy2 = sp.tile([HW, 1024], F16, tag=f"dy2{h}")
                for b in range(2):
                    sl = slice(512 * b, 512 * (b + 1))
                    pK = ppH.tile([HW, 512], F32, tag="psH")
                    nc.tensor.matmul(pK[:], lhsT=ones16_s[0:1, 0:HW], rhs=kcx16[0:1, sl],
                                     start=True, stop=True)
                    nc.scalar.activation(dx2[:, sl], pK[:], AF.Square,
                                         bias=nqc2[:, 0:1])
                    pK2 = ppH.tile([HW, 512], F32, tag="psH")
                    nc.tensor.matmul(pK2[:], lhsT=ones16_s[0:1, 0:HW], rhs=kcy16[0:1, sl],
                                     start=True, stop=True)
                    nc.scalar.activation(dy2[:, sl], pK2[:], AF.Square,
                                         bias=nqc2[:, 1:2])
                nc.vector.tensor_add(dx2[:], dx2[:], dy2[:])
                nkk = sp.tile([HW, 1024], F16, tag=f"nkk{h}")
                nc.scalar.activation(nkk[:], dx2[:], AF.Sqrt)
                V80 = sp.tile([80, 1024], F16, tag=f"V80{h}")
                for b in range(2):
                    sl = slice(512 * b, 512 * (b + 1))
                    pV = ppH.tile([80, 512], F32, tag="psH")
                    nc.tensor.matmul(pV[:], lhsT=permV_s[:], rhs=nkk[:, sl],
                                     start=True, stop=False)
                    nc.tensor.matmul(pV[:], lhsT=insV_s[:], rhs=nkps16[0:1, sl],
                                     start=False, stop=True)
                    nc.scalar.activation(V80[:, sl], pV[:], AF.Copy)
                return V80

            V800 = nkk_v(0, nqc20)

            # ---------------- phase 2 ----------------
            gin = ppP.tile([128, 1024], F32, tag="gA")
            hin = ppP.tile([128, 1024], F32, tag="gB")
            ph2 = sp.tile([128, 4, 1024], F8, tag="ph2")
            pg2 = sp.tile([128, 4, 1024], F16, tag="pg2")

            A, Dv, P = nc.scalar, nc.vector, nc.gpsimd
            hlo_eng = [A, Dv] * 8
            hhi_eng = [Dv, A] * 8
            g_eng = ([Dv, Dv, P] * 6)[:HG]

            def _relu(eng, dst, src, bcol):
                if eng is A:
                    nc.scalar.activation(dst, src, AF.Relu, bias=bcol)
                else:
                    eng.tensor_scalar(dst, src, bcol, 0.0, OP.add, OP.max)

            def phase2_half(h, AT4h, T14h, V80):
                for gp in range(HG):
                    g = HG * h + gp
                    s = g % 4
                    psa = ppH.tile([128, 512], F32, tag="psH")
                    nc.tensor.matmul(psa[:], lhsT=hpatV_s[:, gp, :],
                                     rhs=V80[:, 0:512], start=True, stop=True)
                    psb = ppH.tile([128, 512], F32, tag="psH")
                    nc.tensor.matmul(psb[:], lhsT=hpatV_s[:, gp, :],
                                     rhs=V80[:, 512:1024], start=True, stop=True)
                    _relu(hlo_eng[gp], ph2[:, s, 0:512], psa[:], T14h[:, gp:gp + 1])
                    _relu(hhi_eng[gp], ph2[:, s, 512:1024], psb[:], T14h[:, gp:gp + 1])
                    _relu(g_eng[gp], pg2[:, s, :], B4[:], AT4h[:, gp:gp + 1])
                    for b in range(2):
                        sl = slice(512 * b, 512 * (b + 1))
                        nc.tensor.matmul(gin[:, sl], lhsT=ccg_s[:, g, :],
                                         rhs=pg2[:, s, sl],
                                         start=(g == 0), stop=(g == 2 * HG - 1))
                    if s % 2 == 1:
                        pr = g // 2
                        t0 = 2 * (pr % 2)
                        for b in range(2):
                            sl = slice(512 * b, 512 * (b + 1))
                            nc.tensor.matmul(hin[:, sl], lhsT=cch_s[:, pr, :, :],
                                             rhs=ph2[:, t0:t0 + 2, sl], perf_mode=DR,
                                             start=(pr == 0), stop=(pr == HG - 1))

            phase2_half(0, AT40, T140, V800)

            # logits (needs late consts; runs in the h0 drain window)
            lraw = sp.tile([128, 1024], F16, tag="lraw")
            for b in range(2):
                psl = ppH.tile([128, 512], F32, tag="psH")
                for c in range(2):
                    nc.tensor.matmul(psl[:], lhsT=qT3_s[:, c, :],
                                     rhs=keyT3_s[:, c, 512 * b:512 * (b + 1)],
                                     start=(c == 0), stop=(c == 1))
                nc.scalar.activation(lraw[:, 512 * b:512 * (b + 1)], psl[:],
                                     AF.Copy, scale=1.0 / 16.0)

            AT41, T141, nqc21 = q_prep(1, psq1)
            V801 = nkk_v(1, nqc21)
            phase2_half(1, AT41, T141, V801)

            # ---------------- tail: R, softmax, attention, FFN, LN ----------
            rg = sp.tile([128, 1024], F16, tag="rg")
            nc.scalar.activation(rg[:], gin[:], AF.Relu, bias=g2b_s[:])
            rh = sp.tile([128, 1024], F16, tag="rh")
            nc.vector.tensor_scalar(rh[:], hin[:], h2b_s[:], 0.0, OP.add, OP.max)
            lg = sp.tile([128, 1024], F16, tag="lg")
            nc.vector.tensor_mul(lg[:], rg[:], rh[:])
            nc.vector.tensor_mul(lg[:], lg[:], lraw[:])
            mx = sp.tile([128, 1], F32, tag="mx")
            nc.vector.tensor_reduce(mx[:], lg[:], mybir.AxisListType.X, OP.max)
            nmx = sp.tile([128, 1], F32, tag="nmx")
            nc.vector.tensor_scalar_mul(nmx[:], mx[:], -1.0)
            pexp = sp.tile([128, 1024], F16, tag="pexp")
            sume = sp.tile([128, 1], F32, tag="sume")
            nc.scalar.activation(pexp[:], lg[:], AF.Exp, bias=nmx[:], accum_out=sume[:])
            rsum = sp.tile([128, 1], F32, tag="rsum")
            nc.vector.reciprocal(rsum[:], sume[:])
            pT = sp.tile([128, 8, 128], F16, tag="pT")
            nc.scalar.dma_start_transpose(pT[:], pexp[:])
            attn = ppH.tile([128, 256], F32, tag="psH")
            for t in range(8):
                nc.tensor.matmul(attn[:], lhsT=pT[:, t, :], rhs=key3_s[:, t, :],
                                 start=(t == 0), stop=(t == 7))
            xpre = sp.tile([128, 256], F32, tag="xpre")
            nc.vector.scalar_tensor_tensor(xpre[:], attn[:], rsum[:], qsbh_s[:],
                                           op0=OP.mult, op1=OP.add)

            def layer_norm(src, gt, bt, tag):
                m = sp.tile([128, 1], F32, tag=f"m_{tag}")
                nc.vector.tensor_reduce(m[:], src[:], mybir.AxisListType.X, OP.add)
                nc.vector.tensor_scalar_mul(m[:], m[:], -1.0 / 256.0)
                xc = sp.tile([128, 256], F32, tag=f"xc_{tag}")
                nc.vector.tensor_scalar_add(xc[:], src[:], m[:])
                var = sp.tile([128, 1], F32, tag=f"v_{tag}")
                xc2 = sp.tile([128, 256], F32, tag=f"xc2_{tag}")
                nc.vector.scalar_tensor_tensor(xc2[:], xc[:], 1.0, xc[:],
                                               op0=OP.mult, op1=OP.mult, accum_out=var[:])
                lnv = sp.tile([128, 1], F32, tag=f"lv_{tag}")
                nc.scalar.activation(lnv[:], var[:], AF.Ln, scale=1.0 / 256.0, bias=eps_s[:])
                rstd = sp.tile([128, 1], F32, tag=f"rs_{tag}")
                nc.scalar.activation(rstd[:], lnv[:], AF.Exp, scale=-0.5)
                y = sp.tile([128, 256], F32, tag=f"y_{tag}")
                nc.vector.scalar_tensor_tensor(y[:], xc[:], rstd[:], gt[:],
                                               op0=OP.mult, op1=OP.mult)
                nc.vector.tensor_add(y[:], y[:], bt[:])
                return y

            x1 = layer_norm(xpre, l1g_s, l1b_s, "ln1")
            x1h = sp.tile([128, 256], F16, tag="x1h")
            nc.vector.tensor_copy(x1h[:], x1[:])
            xT = sp.tile([128, 2, 128], F16, tag="xT")
            nc.scalar.dma_start_transpose(xT[:], x1h[:])
            hT = sp.tile([128, 8, 128], F16, tag="hT")
            for t in range(8):
                psh = ppH.tile([128, 128], F32, tag="psH")
                for c in range(2):
                    nc.tensor.matmul(psh[:], lhsT=f1w_s[:, c, t, :], rhs=xT[:, c, :],
                                     start=(c == 0), stop=(c == 1))
                if t % 2 == 0:
                    nc.scalar.activation(hT[:, t, :], psh[:], AF.Relu, bias=f1b_s[:, t:t + 1])
                else:
                    nc.vector.tensor_scalar(hT[:, t, :], psh[:], f1b_s[:, t:t + 1],
                                            0.0, OP.add, OP.max)
            pso = ppH.tile([128, 256], F32, tag="psH")
            for t in range(8):
                nc.tensor.matmul(pso[:], lhsT=hT[:, t, :], rhs=f2w_s[:, t, :],
                                 start=(t == 0), stop=(t == 7))
            y2 = sp.tile([128, 256], F32, tag="y2")
            nc.vector.tensor_add(y2[:], pso[:], x1[:])
            nc.vector.tensor_add(y2[:], y2[:], f2b_s[:])
            x2 = layer_norm(y2, l2g_s, l2b_s, "ln2")
            nc.scalar.dma_start(out=out_d[:], in_=x2[:])

    _split_multiwaits(nc)
    return nc


# ---------------------------------------------------------------------------
def prep_inputs(inp):
    f32 = np.float32
    f16 = np.float16
    q_asn = np.asarray(inp["q_assignments"], f32)
    k_asn = np.asarray(inp["k_assignments"], f32)
    query = np.asarray(inp["query"], f32)
    key = np.asarray(inp["key_emb"], f32)

    def xfeat(coords, iso):
        oh = np.zeros((N, NISO), f32)
        oh[np.arange(N), np.asarray(iso) - 1] = 1.0
        x = np.concatenate([oh, np.ones((N, 1), f32), np.asarray(coords, f32),
                            np.zeros((N, 1), f32)], axis=1)
        return np.ascontiguousarray(
            x.reshape(NCH, 128, XC).transpose(1, 0, 2)).astype(NP8)

    qx8 = xfeat(inp["q_coords"], inp["q_iso"])
    kx8 = xfeat(inp["k_coords"], inp["k_iso"])

    g1 = np.asarray(inp["g1_w"], f32)
    g1b = np.asarray(inp["g1_b"], f32)
    g2 = np.asarray(inp["g2_w"], f32)[:, 0]
    g2b = float(np.asarray(inp["g2_b"], f32)[0])
    h1 = np.asarray(inp["h1_w"], f32)
    h1b = np.asarray(inp["h1_b"], f32)
    h2 = np.asarray(inp["h2_w"], f32)[:, 0]
    h2b = float(np.asarray(inp["h2_b"], f32)[0])

    ag, sg = np.abs(g2), np.sign(g2).astype(f32)
    ah, sh = np.abs(h2), np.sign(h2).astype(f32)

    g1ab = np.concatenate([g1[:12] * ag[None, :], (g1b * ag)[None, :]], axis=0)
    g1b4 = np.tile(g1[12:] * ag[None, :], (1, 4))

    hpatV = np.zeros((80, HG, 128), f32)
    for gp in range(HG):
        for ii in range(4):
            hpatV[5 * gp + ii, gp, 32 * ii:32 * (ii + 1)] = h1[0] * ah
            hpatV[5 * gp + 4, gp, 32 * ii:32 * (ii + 1)] = h1[2] * ah
    wq = (h1[1] * ah)[None, :].astype(f32)
    b1r = (h1b * ah)[None, :].astype(f32)

    # H contract: DoubleRow pattern per global pair pr (groups 2pr, 2pr+1),
    # writing gin/hin columns 4g..4g+3.
    cchDR = np.zeros((128, HG, 2, 128), f32)
    for pr in range(HG):
        for t in range(2):
            gg = 2 * pr + t
            for ii in range(4):
                for c in range(32):
                    cchDR[32 * ii + c, pr, t, 4 * gg + ii] = sh[c]
    cchDR = cchDR.astype(NP8)

    ccg16 = np.zeros((128, 2 * HG, 128), f32)
    for g in range(2 * HG):
        for ii in range(4):
            for c in range(32):
                ccg16[32 * ii + c, g, 4 * g + ii] = sg[c]
    ccg16 = ccg16.astype(f16)

    permV = np.zeros((HW, 80), f32)
    for i in range(HW):
        permV[i, 5 * (i // 4) + (i % 4)] = 1.0
    insV = np.zeros((1, 80), f32)
    insV[0, 4::5] = 1.0

    sens = np.asarray(inp["sensor_coords"], f32)[None, :]
    sensr = np.tile(sens, (HW, 1))

    f1wt = np.ascontiguousarray(
        np.asarray(inp["ffn1_w"], f32).reshape(2, 128, 8, 128).transpose(1, 0, 2, 3)).astype(f16)
    f1b = np.ascontiguousarray(np.asarray(inp["ffn1_b"], f32).reshape(8, 128).T)
    f2wt = np.ascontiguousarray(
        np.asarray(inp["ffn2_w"], f32).reshape(8, 128, 256).transpose(1, 0, 2)).astype(f16)
    f2br = np.tile(np.asarray(inp["ffn2_b"], f32)[None, :], (128, 1))
    l1g = np.tile(np.asarray(inp["ln1_g"], f32)[None, :], (128, 1))
    l1b = np.tile(np.asarray(inp["ln1_b"], f32)[None, :], (128, 1))
    l2g = np.tile(np.asarray(inp["ln2_g"], f32)[None, :], (128, 1))
    l2b = np.tile(np.asarray(inp["ln2_b"], f32)[None, :], (128, 1))

    keyT3 = np.ascontiguousarray(key.T.reshape(2, 128, 1024).transpose(1, 0, 2)).astype(f16)
    key3 = np.ascontiguousarray(key.reshape(8, 128, 256).transpose(1, 0, 2)).astype(f16)

    shared = {
        "qx8": qx8, "kx8": kx8,
        "keyT3": keyT3, "key3": key3,
        "g1ab": g1ab, "g1b4": g1b4.astype(f16),
        "hpatV": hpatV.astype(f16), "wq_h": wq, "b1_h": b1r,
        "g2b": np.full((128, 1), g2b, f32), "h2b": np.full((128, 1), h2b, f32),
        "cchDR": cchDR, "ccg16": ccg16,
        "permV": permV.astype(f16), "insV": insV.astype(f16),
        "ones16": np.ones((1, 128), f16), "ones32": np.ones((1, 64), f32),
        "i64f": np.eye(64, dtype=f32), "i128f": np.eye(128, dtype=f32),
        "i128h": np.eye(128, dtype=f16), "sensr128": np.tile(sens, (128, 1)),
        "sens": sens, "sensr": sensr,
        "f1w": f1wt, "f1b": f1b, "f2w": f2wt, "f2br": f2br,
        "l1g": l1g, "l1b": l1b, "l2g": l2g, "l2b": l2b,
        "epsc": np.full((128, 1), 1e-6, f32),
    }

    in_maps = []
    for m in range(NCORES):
        sl = slice(m * KSL, (m + 1) * KSL)
        qa = (q_asn[:, sl] * ASCALE).astype(NP8)
        ka = (k_asn[:, sl] * ASCALE).astype(NP8)
        qa8 = np.ascontiguousarray(
            qa.reshape(NCH, 128, NH, HW).transpose(1, 2, 0, 3))
        ka8 = np.ascontiguousarray(
            ka.reshape(NCH, 128, 128).transpose(1, 0, 2))
        qT3 = np.ascontiguousarray(
            query[sl].T.reshape(2, 128, 128).transpose(1, 0, 2)).astype(f16)
        im = dict(shared)
        im.update({
            "qa8": qa8, "ka8": ka8, "qT3": qT3,
            "q_sbh": np.ascontiguousarray(query[sl]),
        })
        in_maps.append(im)
    return in_maps


def kernel(**inputs) -> np.ndarray:
    if "nc" not in _cache:
        _cache["nc"] = build_program()
    nc = _cache["nc"]
    in_maps = prep_inputs(inputs)
    res = run_bass_kernel_spmd(nc, in_maps, list(range(NCORES)))
    return np.concatenate([res.results[m]["out"] for m in range(NCORES)], axis=0)


# revision 9
# speedup vs baseline: 1.9320x; 1.0764x over previous
"""Trainium2 Bass kernel for nn_CustomAttentionLayer (sparse_attention).

Strategy (8 NeuronCores, SPMD single launch), v3:
 - Shard the K=1024 query-cluster axis: core m owns rows [128m, 128m+128).
 - Phase 1 (DMA-bound): assignments host-cast to fp8e4 (scaled x256),
   streamed through the PE with the small fp8 feature matrix
   X = [one_hot(iso) | ones | coords] as the moving operand (16 cycles per
   128-pixel chunk). SP queue order: kx, ka, qx, qa, then heavy consts so
   the k-side finishes (and the AllGather starts) as early as possible.
 - Phase 2: R = G*H with (4i x 32c) partition packing, q-side split in two
   64-row halves for overlap. H hidden built on the PE from an interleaved
   V tile [nkk x4 | n_kps] per group (zero-padded (80,128) lhsT); relu off
   PSUM on ACT/DVE with per-partition T14 bias -> fp8, contracted in
   group-pairs with DoubleRow fp8 matmuls. G hidden relu(B4 + AT4 col) in
   fp16 on DVE (4x mode) / Pool, contracted per group in fp16.
   4 hidden-activation slots + 4 rotating half-width H-build psum banks
   decouple the PE / relu / contract pipeline stages.
 - Tail: logits*R, softmax (DMA-transpose), attention, FFN, layernorms.
"""
import numpy as np
import ml_dtypes

import concourse.bass as bass
import concourse.mybir as mybir
import concourse.tile as tile
from concourse.bass_utils import run_bass_kernel_spmd

F32 = mybir.dt.float32
F16 = mybir.dt.float16
F8 = mybir.dt.float8e4
AF = mybir.ActivationFunctionType
OP = mybir.AluOpType
DR = mybir.MatmulPerfMode.DoubleRow
NP8 = ml_dtypes.float8_e4m3

NCORES = 8
K, D, N, NISO = 1024, 256, 65536, 12
KSL = K // NCORES           # 128 rows per core
NCH = N // 128              # 512 contraction chunks
GRP = 64                    # chunks per DMA group
NGRP = NCH // GRP           # 8
XC = 16                     # X feature cols: [oh(12) | ones | cx | cy | 0]
ASCALE = 256.0              # host-side scale on assignments before fp8 cast
NH = 2                      # q-side halves
HW = KSL // NH              # 64 i-rows per half
HG = 16                     # phase-2 groups per half

_cache = {}


# ---------------------------------------------------------------------------
# walrus in this container rejects >1 sync wait per instruction; split extras
# onto single-wait NOPs on the same engine right before the instruction.
def _split_multiwaits(nc):
    ctr = 0
    for f in nc.m.functions:
        for bb in f.blocks:
            for inst in list(bb.instructions):
                si = inst.sync_info
                if si is None:
                    continue
                waits = list(si.on_wait)
                if len(waits) <= 1:
                    continue
                si.on_wait = [waits[-1]]
                pos = None
                for j, cur in enumerate(bb.instructions):
                    if cur.name == inst.name:
                        pos = j
                        break
                assert pos is not None
                for k2, w in enumerate(waits[:-1]):
                    nop = mybir.InstNoOp(
                        name=f"wsplit-{ctr}",
                        sync_info=mybir.SyncInfo(on_wait=[w], on_update=[]),
                        engine=inst.engine,
                        bass_nofuse=True,
                    )
                    ctr += 1
                    nc.register_instruction(nop)
                    bb.instructions.insert(pos + k2, nop)
    return ctr


def build_program():
    nc = bass.Bass()

    def din(name, shape, dt=F32):
        return nc.dram_tensor(name, list(shape), dt, kind="ExternalInput")

    # big streams
    ka8_h = din("ka8", (128, NCH, 128), F8)
    qa8_h = din("qa8", (128, NH, NCH, HW), F8)
    kx8_h = din("kx8", (128, NCH, XC), F8)
    qx8_h = din("qx8", (128, NCH, XC), F8)
    # heavy consts (loaded late on the SP queue)
    qT3_h = din("qT3", (128, 2, 128), F16)
    keyT3_h = din("keyT3", (128, 2, 1024), F16)
    key3_h = din("key3", (128, 8, 256), F16)
    q_sb_h = din("q_sbh", (128, 256))
    f1w_h = din("f1w", (128, 2, 8, 128), F16)
    f1b_h = din("f1b", (128, 8))
    f2w_h = din("f2w", (128, 8, 256), F16)
    f2b_h = din("f2br", (128, 256))
    l1g_h = din("l1g", (128, 256))
    l1b_h = din("l1b", (128, 256))
    l2g_h = din("l2g", (128, 256))
    l2b_h = din("l2b", (128, 256))
    cchDR_h = din("cchDR", (128, HG, 2, 128), F8)    # H contract DR patterns
    ccg16_h = din("ccg16", (128, 2 * HG, 128), F16)  # G contract per group
    # small consts (scalar queue, early)
    eps_h = din("epsc", (128, 1))
    g1ab_h = din("g1ab", (13, 32))
    g1b4_h = din("g1b4", (12, 128), F16)
    hpatV_h = din("hpatV", (80, HG, 128), F16)
    wq_h = din("wq_h", (1, 32))
    b1_h = din("b1_h", (1, 32))
    g2b_h = din("g2b", (128, 1))
    h2b_h = din("h2b", (128, 1))
    permV_h = din("permV", (HW, 80), F16)
    insV_h = din("insV", (1, 80), F16)
    ones16_h = din("ones16", (1, 128), F16)
    ones32_h = din("ones32", (1, 64))
    i64f_h = din("i64f", (64, 64))
    i128f_h = din("i128f", (128, 128))
    i128h_h = din("i128h", (128, 128), F16)
    sensr128_h = din("sensr128", (128, 2))
    sens_h = din("sens", (1, 2))
    sensr_h = din("sensr", (64, 2))
    onec_h = din("onec", (64, 1))

    out_d = nc.dram_tensor("out", [128, 256], F32, kind="ExternalOutput")
    kside_d = nc.dram_tensor("kside", [16, 128], F16)
    kall_d = nc.dram_tensor("kall", [128, 128], F16, addr_space="Shared")

    with tile.TileContext(nc) as tc:
        with (
            tc.tile_pool(name="consts", bufs=1) as cp,
            tc.tile_pool(name="astream", bufs=3) as ap,
            tc.tile_pool(name="sb", bufs=1) as sp,
            tc.tile_pool(name="ppP", bufs=1, space="PSUM") as ppP,
            tc.tile_pool(name="ppH", bufs=4, space="PSUM") as ppH,
        ):
            def cload(dram, eng, tag=None):
                t = cp.tile(list(dram.shape), dram.dtype, tag=tag or dram.name)
                eng.dma_start(out=t[:], in_=dram[:])
                return t

            sc = nc.gpsimd
            # small consts via SWDGE so HWDGE stays free for the A stream
            eps_s = cload(eps_h, sc)
            g1ab_s = cload(g1ab_h, sc)
            g1b4_s = cload(g1b4_h, sc)
            hpatV_s = cload(hpatV_h, sc)
            wq_s = cload(wq_h, sc)
            b1_s = cload(b1_h, sc)
            g2b_s = cload(g2b_h, sc)
            h2b_s = cload(h2b_h, sc)
            permV_s = cload(permV_h, sc)
            insV_s = cload(insV_h, sc)
            ones16_s = cload(ones16_h, sc)
            ones32_s = cload(ones32_h, sc)
            i64f_s = cload(i64f_h, sc)
            i128f_s = cload(i128f_h, sc)
            i128h_s = cload(i128h_h, sc)
            sensr128_s = cload(sensr128_h, sc)
            sens_s = cload(sens_h, sc)
            sensr_s = cload(sensr_h, sc)
            onec_s = cload(onec_h, sc)

            # ---------------- phase 1: k-side reduction ----------------
            kx_s = cload(kx8_h, nc.sync)
            psk = ppP.tile([128, XC], F32, tag="gB")        # shares bank w/ hin
            for g in range(NGRP):
                at = ap.tile([128, GRP, 128], F8, tag="ka")
                nc.sync.dma_start(out=at[:], in_=ka8_h[:, g * GRP:(g + 1) * GRP, :])
                for c in range(GRP):
                    nc.tensor.matmul(
                        psk[:], lhsT=at[:, c, :], rhs=kx_s[:, g * GRP + c, :],
                        start=(g == 0 and c == 0),
                        stop=(g == NGRP - 1 and c == GRP - 1),
                    )
            # pre-normalize this core's k-slab (tiny per-partition col ops)
            hp = tc.high_priority()
            hp.__enter__()
            ksb = sp.tile([128, XC], F32, tag="ksb")
            nc.vector.tensor_copy(ksb[:], psk[:])
            rkc = sp.tile([128, 1], F32, tag="rkc")
            nc.vector.tensor_scalar_add(rkc[:], ksb[:, 12:13], ASCALE * 1e-6)
            nc.vector.reciprocal(rkc[:], rkc[:])
            kprep = sp.tile([128, XC], F16, tag="kprep")
            nc.vector.tensor_scalar_mul(kprep[:, 0:12], ksb[:, 0:12], rkc[:])
            kc2 = sp.tile([128, 2], F32, tag="kc2")
            nc.vector.tensor_scalar_mul(kc2[:], ksb[:, 13:15], rkc[:])
            nc.vector.tensor_copy(kprep[:, 13:15], kc2[:])
            dk2 = sp.tile([128, 2], F32, tag="dk2")
            nc.vector.tensor_sub(dk2[:], kc2[:], sensr128_s[:])
            nc.vector.tensor_mul(dk2[:], dk2[:], dk2[:])
            nkp_c = sp.tile([128, 1], F32, tag="nkp_c")
            nc.vector.tensor_reduce(nkp_c[:], dk2[:], mybir.AxisListType.X, OP.add)
            nc.scalar.activation(kprep[:, 12:13], nkp_c[:], AF.Sqrt)
            kc2sq = sp.tile([128, 2], F32, tag="kc2sq")
            nc.vector.tensor_mul(kc2sq[:], kc2[:], kc2[:])
            kn2c = sp.tile([128, 1], F32, tag="kn2c")
            nc.vector.tensor_reduce(kn2c[:], kc2sq[:], mybir.AxisListType.X, OP.add)
            nc.vector.tensor_copy(kprep[:, 15:16], kn2c[:])
            pskT = ppH.tile([XC, 128], F16, tag="psH")
            nc.tensor.transpose(pskT[:], kprep[:], i128h_s[:])
            ksbT = sp.tile([XC, 128], F16, tag="ksbT")
            nc.vector.tensor_copy(ksbT[:], pskT[:])
            nc.sync.dma_start(out=kside_d[:], in_=ksbT[:])
            nc.gpsimd.collective_compute(
                "AllGather", OP.bypass,
                replica_groups=[list(range(NCORES))],
                ins=[kside_d[:]],
                outs=[kall_d[:]],
            )
            kview = kall_d.rearrange("(g c) k -> c g k", c=16)
            kxy3 = sp.tile([3, 1024], F16, tag="kxy3")
            nc.scalar.dma_start(out=kxy3[:].rearrange("c (g k) -> c g k", g=8),
                                in_=kview[13:16, :, :])
            dkpn = sp.tile([12, 1024], F16, tag="dkpn")
            nc.scalar.dma_start(out=dkpn[:].rearrange("c (g k) -> c g k", g=8),
                                in_=kview[0:12, :, :])
            nkps16 = sp.tile([1, 1024], F16, tag="nkps16")
            nc.sync.dma_start(out=nkps16[:].rearrange("c (g k) -> c g k", g=8),
                              in_=kview[12:13, :, :])
            hp.__exit__(None, None, None)

            # ---------------- q-side half-0 stream ----------------
            qx_s = cload(qx8_h, nc.sync)
            psq0 = ppH.tile([HW, XC], F32, tag="psH")
            for g in range(NGRP):
                at = ap.tile([128, GRP, HW], F8, tag="qa")
                nc.sync.dma_start(out=at[:], in_=qa8_h[:, 0, g * GRP:(g + 1) * GRP, :])
                for c in range(GRP):
                    nc.tensor.matmul(
                        psq0[:], lhsT=at[:, c, :], rhs=qx_s[:, g * GRP + c, :],
                        start=(g == 0 and c == 0),
                        stop=(g == NGRP - 1 and c == GRP - 1),
                    )

            def q_prep(h, psq):
                """q-side stats for half h -> (AT4h, T14h, nqc2)."""
                qT = sp.tile([HW, XC], F32, tag=f"qT{h}")
                nc.vector.tensor_copy(qT[:], psq[:])
                rq = sp.tile([HW, 1], F32, tag=f"rq{h}")
                nc.vector.tensor_scalar_add(rq[:], qT[:, 12:13], ASCALE * 1e-6)
                nc.vector.reciprocal(rq[:], rq[:])
                qn = sp.tile([HW, 13], F32, tag=f"qn{h}")
                nc.vector.tensor_scalar_mul(qn[:], qT[:, 0:13], rq[:])
                qc2 = sp.tile([HW, 2], F32, tag=f"qc2{h}")
                nc.vector.tensor_scalar_mul(qc2[:], qT[:, 13:15], rq[:])
                nqc2 = sp.tile([HW, 2], F32, tag=f"nqc2{h}")
                nc.vector.tensor_scalar_mul(nqc2[:], qc2[:], -1.0)
                d2 = sp.tile([HW, 2], F32, tag=f"d2{h}")
                nc.vector.tensor_sub(d2[:], qc2[:], sensr_s[:])
                nc.vector.tensor_mul(d2[:], d2[:], d2[:])
                nks_c = sp.tile([HW, 1], F32, tag=f"nksc{h}")
                nc.vector.tensor_reduce(nks_c[:], d2[:], mybir.AxisListType.X, OP.add)
                nc.scalar.activation(nks_c[:], nks_c[:], AF.Sqrt)
                pqnT = ppH.tile([13, HW], F32, tag="psH")
                nc.tensor.transpose(pqnT[:], qn[:], i64f_s[:])
                qsT = sp.tile([13, HW], F32, tag=f"qsT{h}")
                nc.vector.tensor_copy(qsT[:], pqnT[:])
                pnksT = ppH.tile([1, HW], F32, tag="psH")
                nc.tensor.transpose(pnksT[:], nks_c[:], i64f_s[:])
                nks_r = sp.tile([1, HW], F32, tag=f"nksr{h}")
                nc.vector.tensor_copy(nks_r[:], pnksT[:])
                psA = ppH.tile([128, HG], F32, tag="psH")
                for ii in range(4):
                    nc.tensor.matmul(psA[32 * ii:32 * (ii + 1), :], lhsT=g1ab_s[:],
                                     rhs=qsT[:, ii::4], start=True, stop=True,
                                     tile_position=(0, 32 * ii))
                AT4h = sp.tile([128, HG], F32, tag=f"AT4{h}")
                nc.vector.tensor_copy(AT4h[:], psA[:])
                psT1 = ppH.tile([128, HG], F32, tag="psH")
                for ii in range(4):
                    nc.tensor.matmul(psT1[32 * ii:32 * (ii + 1), :], lhsT=wq_s[:],
                                     rhs=nks_r[0:1, ii::4], start=True, stop=False,
                                     tile_position=(0, 32 * ii))
                    nc.tensor.matmul(psT1[32 * ii:32 * (ii + 1), :], lhsT=b1_s[:],
                                     rhs=ones32_s[0:1, ii::4], start=False, stop=True,
                                     tile_position=(0, 32 * ii))
                T14h = sp.tile([128, HG], F32, tag=f"T14{h}")
                nc.vector.tensor_copy(T14h[:], psT1[:])
                # nkk-matmul ingredients: qn2e col and (3,64) lhsT [-2qc | 1]
                qc2sq = sp.tile([HW, 2], F32, tag=f"qc2sq{h}")
                nc.vector.tensor_mul(qc2sq[:], qc2[:], qc2[:])
                qn2e = sp.tile([HW, 1], F32, tag=f"qn2e{h}")
                nc.vector.tensor_reduce(qn2e[:], qc2sq[:], mybir.AxisListType.X, OP.add)
                nc.vector.tensor_scalar_add(qn2e[:], qn2e[:], 1e-4)
                qext = sp.tile([HW, 3], F32, tag=f"qext{h}")
                nc.vector.tensor_scalar_mul(qext[:, 0:2], qc2[:], -2.0)
                nc.vector.tensor_copy(qext[:, 2:3], onec_s[:])
                pqeT = ppH.tile([3, HW], F32, tag="psH")
                nc.tensor.transpose(pqeT[:], qext[:], i64f_s[:])
                qn3T = sp.tile([3, HW], F16, tag=f"qn3T{h}")
                nc.vector.tensor_copy(qn3T[:], pqeT[:])
                return AT4h, T14h, qn3T, qn2e

            AT40, T140, qn3T0, qn2e0 = q_prep(0, psq0)

            # ---------------- k-side prep (after collective): just B4 -------
            B4 = sp.tile([128, 1024], F16, tag="B4")
            for b in range(2):
                sl = slice(512 * b, 512 * (b + 1))
                psB = ppH.tile([128, 512], F32, tag="psH")
                nc.tensor.matmul(psB[:], lhsT=g1b4_s[:], rhs=dkpn[0:12, sl],
                                 start=True, stop=True)
                nc.vector.tensor_copy(B4[:, sl], psB[:])

            # ---------------- q-side half-1 stream ----------------
            psq1 = ppP.tile([HW, XC], F32, tag="gA")        # shares bank w/ gin
            for g in range(NGRP):
                at = ap.tile([128, GRP, HW], F8, tag="qa")
                nc.sync.dma_start(out=at[:], in_=qa8_h[:, 1, g * GRP:(g + 1) * GRP, :])
                for c in range(GRP):
                    nc.tensor.matmul(
                        psq1[:], lhsT=at[:, c, :], rhs=qx_s[:, g * GRP + c, :],
                        start=(g == 0 and c == 0),
                        stop=(g == NGRP - 1 and c == GRP - 1),
                    )

            # heavy consts stream behind the last qa group on SP
            cch_s = cload(cchDR_h, nc.sync)
            ccg_s = cload(ccg16_h, nc.sync)
            qT3_s = cload(qT3_h, nc.sync)
            keyT3_s = cload(keyT3_h, nc.sync)
            key3_s = cload(key3_h, nc.sync)
            f1w_s = cload(f1w_h, nc.sync)
            f2w_s = cload(f2w_h, nc.sync)
            qsbh_s = cload(q_sb_h, nc.sync)
            f1b_s = cload(f1b_h, nc.sync)
            f2b_s = cload(f2b_h, nc.sync)
            l1g_s = cload(l1g_h, nc.sync)
            l1b_s = cload(l1b_h, nc.sync)
            l2g_s = cload(l2g_h, nc.sync)
            l2b_s = cload(l2b_h, nc.sync)

            def nkk_v(h, qn3T, qn2e):
                """nkk via nkk^2 = qn2_i + (-2qc.kc + kn2_j), then V80."""
                nkk = sp.tile([HW, 1024], F16, tag=f"nkk{h}")
                V80 = sp.tile([80, 1024], F16, tag=f"V80{h}")
                for b in range(2):
                    sl = slice(512 * b, 512 * (b + 1))
                    pS = ppH.tile([HW, 512], F32, tag="psH")
                    nc.tensor.matmul(pS[:], lhsT=qn3T[:], rhs=kxy3[0:3, sl],
                                     start=True, stop=True)
                    nc.scalar.activation(nkk[:, sl], pS[:], AF.Sqrt, bias=qn2e[:, 0:1])
                for b in range(2):
                    sl = slice(512 * b, 512 * (b + 1))
                    pV = ppH.tile([80, 512], F32, tag="psH")
                    nc.tensor.matmul(pV[:], lhsT=permV_s[:], rhs=nkk[:, sl],
                                     start=True, stop=False)
                    nc.tensor.matmul(pV[:], lhsT=insV_s[:], rhs=nkps16[0:1, sl],
                                     start=False, stop=True)
                    if b == 0:
                        nc.vector.tensor_copy(V80[:, sl], pV[:])
                    else:
                        nc.scalar.activation(V80[:, sl], pV[:], AF.Copy)
                return V80

            V800 = nkk_v(0, qn3T0, qn2e0)

            # ---------------- phase 2 ----------------
            gin = ppP.tile([128, 1024], F32, tag="gA")
            hin = ppP.tile([128, 1024], F32, tag="gB")
            ph2 = sp.tile([128, 4, 1024], F8, tag="ph2")
            pg2 = sp.tile([128, 4, 1024], F16, tag="pg2")

            A, Dv, P = nc.scalar, nc.vector, nc.gpsimd
            hlo_eng = [A, Dv] * 8
            hhi_eng = [Dv, A] * 8
            g_eng = ([Dv, Dv, P] * 6)[:HG]

            def _relu(eng, dst, src, bcol):
                if eng is A:
                    nc.scalar.activation(dst, src, AF.Relu, bias=bcol)
                else:
                    eng.tensor_scalar(dst, src, bcol, 0.0, OP.add, OP.max)

            def phase2_half(h, AT4h, T14h, V80, mid=None):
                for gp in range(HG):
                    if gp == 8 and mid is not None:
                        mid()
                    g = HG * h + gp
                    s = g % 4
                    psa = ppH.tile([128, 512], F32, tag="psH")
                    nc.tensor.matmul(psa[:], lhsT=hpatV_s[:, gp, :],
                                     rhs=V80[:, 0:512], start=True, stop=True)
                    psb = ppH.tile([128, 512], F32, tag="psH")
                    nc.tensor.matmul(psb[:], lhsT=hpatV_s[:, gp, :],
                                     rhs=V80[:, 512:1024], start=True, stop=True)
                    _relu(hlo_eng[gp], ph2[:, s, 0:512], psa[:], T14h[:, gp:gp + 1])
                    _relu(hhi_eng[gp], ph2[:, s, 512:1024], psb[:], T14h[:, gp:gp + 1])
                    _relu(g_eng[gp], pg2[:, s, :], B4[:], AT4h[:, gp:gp + 1])
                    for b in range(2):
                        sl = slice(512 * b, 512 * (b + 1))
                        nc.tensor.matmul(gin[:, sl], lhsT=ccg_s[:, g, :],
                                         rhs=pg2[:, s, sl],
                                         start=(g == 0), stop=(g == 2 * HG - 1))
                    if s % 2 == 1:
                        pr = g // 2
                        t0 = 2 * (pr % 2)
                        for b in range(2):
                            sl = slice(512 * b, 512 * (b + 1))
                            nc.tensor.matmul(hin[:, sl], lhsT=cch_s[:, pr, :, :],
                                             rhs=ph2[:, t0:t0 + 2, sl], perf_mode=DR,
                                             start=(pr == 0), stop=(pr == HG - 1))

            h1ctx = {}

            def h1_prep():
                AT41, T141, qn3T1, qn2e1 = q_prep(1, psq1)
                h1ctx["r"] = (AT41, T141, nkk_v(1, qn3T1, qn2e1))

            phase2_half(0, AT40, T140, V800, mid=h1_prep)

            # logits (needs late consts; runs in the h0 drain window)
            lraw = sp.tile([128, 1024], F16, tag="lraw")
            for b in range(2):
                psl = ppH.tile([128, 512], F32, tag="psH")
                for c in range(2):
                    nc.tensor.matmul(psl[:], lhsT=qT3_s[:, c, :],
                                     rhs=keyT3_s[:, c, 512 * b:512 * (b + 1)],
                                     start=(c == 0), stop=(c == 1))
                nc.scalar.activation(lraw[:, 512 * b:512 * (b + 1)], psl[:],
                                     AF.Copy, scale=1.0 / 16.0)

            AT41, T141, V801 = h1ctx["r"]
            phase2_half(1, AT41, T141, V801)

            # ---------------- tail: R, softmax, attention, FFN, LN ----------
            rg = sp.tile([128, 1024], F16, tag="rg")
            nc.scalar.activation(rg[:], gin[:], AF.Relu, bias=g2b_s[:])
            rh = sp.tile([128, 1024], F16, tag="rh")
            nc.vector.tensor_scalar(rh[:], hin[:], h2b_s[:], 0.0, OP.add, OP.max)
            lg = sp.tile([128, 1024], F16, tag="lg")
            nc.vector.tensor_mul(lg[:], rg[:], rh[:])
            nc.vector.tensor_mul(lg[:], lg[:], lraw[:])
            mx = sp.tile([128, 1], F32, tag="mx")
            nc.vector.tensor_reduce(mx[:], lg[:], mybir.AxisListType.X, OP.max)
            nmx = sp.tile([128, 1], F32, tag="nmx")
            nc.vector.tensor_scalar_mul(nmx[:], mx[:], -1.0)
            pexp = sp.tile([128, 1024], F16, tag="pexp")
            sume = sp.tile([128, 1], F32, tag="sume")
            nc.scalar.activation(pexp[:], lg[:], AF.Exp, bias=nmx[:], accum_out=sume[:])
            rsum = sp.tile([128, 1], F32, tag="rsum")
            nc.vector.reciprocal(rsum[:], sume[:])
            pT = sp.tile([128, 8, 128], F16, tag="pT")
            for t in range(8):
                ptp = ppH.tile([128, 128], F16, tag="psH")
                nc.tensor.transpose(ptp[:], pexp[:, 128 * t:128 * (t + 1)], i128h_s[:])
                if t % 2 == 0:
                    nc.vector.tensor_copy(pT[:, t, :], ptp[:])
                else:
                    nc.scalar.activation(pT[:, t, :], ptp[:], AF.Copy)
            attn = ppH.tile([128, 256], F32, tag="psH")
            for t in range(8):
                nc.tensor.matmul(attn[:], lhsT=pT[:, t, :], rhs=key3_s[:, t, :],
                                 start=(t == 0), stop=(t == 7))
            xpre = sp.tile([128, 256], F32, tag="xpre")
            nc.vector.scalar_tensor_tensor(xpre[:], attn[:], rsum[:], qsbh_s[:],
                                           op0=OP.mult, op1=OP.add)

            def layer_norm(src, gt, bt, tag):
                m = sp.tile([128, 1], F32, tag=f"m_{tag}")
                nc.vector.tensor_reduce(m[:], src[:], mybir.AxisListType.X, OP.add)
                nc.vector.tensor_scalar_mul(m[:], m[:], -1.0 / 256.0)
                xc = sp.tile([128, 256], F32, tag=f"xc_{tag}")
                nc.vector.tensor_scalar_add(xc[:], src[:], m[:])
                var = sp.tile([128, 1], F32, tag=f"v_{tag}")
                xc2 = sp.tile([128, 256], F32, tag=f"xc2_{tag}")
                nc.vector.scalar_tensor_tensor(xc2[:], xc[:], 1.0, xc[:],
                                               op0=OP.mult, op1=OP.mult, accum_out=var[:])
                lnv = sp.tile([128, 1], F32, tag=f"lv_{tag}")
                nc.scalar.activation(lnv[:], var[:], AF.Ln, scale=1.0 / 256.0, bias=eps_s[:])
                rstd = sp.tile([128, 1], F32, tag=f"rs_{tag}")
                nc.scalar.activation(rstd[:], lnv[:], AF.Exp, scale=-0.5)
                y = sp.tile([128, 256], F32, tag=f"y_{tag}")
                nc.vector.scalar_tensor_tensor(y[:], xc[:], rstd[:], gt[:],
                                               op0=OP.mult, op1=OP.mult)
                nc.vector.tensor_add(y[:], y[:], bt[:])
                return y

            x1 = layer_norm(xpre, l1g_s, l1b_s, "ln1")
            x1h = sp.tile([128, 256], F16, tag="x1h")
            nc.vector.tensor_copy(x1h[:], x1[:])
            xT = sp.tile([128, 2, 128], F16, tag="xT")
            for c in range(2):
                pxT = ppH.tile([128, 128], F16, tag="psH")
                nc.tensor.transpose(pxT[:], x1h[:, 128 * c:128 * (c + 1)], i128h_s[:])
                nc.vector.tensor_copy(xT[:, c, :], pxT[:])
            hT = sp.tile([128, 8, 128], F16, tag="hT")
            for t in range(8):
                psh = ppH.tile([128, 128], F32, tag="psH")
                for c in range(2):
                    nc.tensor.matmul(psh[:], lhsT=f1w_s[:, c, t, :], rhs=xT[:, c, :],
                                     start=(c == 0), stop=(c == 1))
                if t % 2 == 0:
                    nc.scalar.activation(hT[:, t, :], psh[:], AF.Relu, bias=f1b_s[:, t:t + 1])
                else:
                    nc.vector.tensor_scalar(hT[:, t, :], psh[:], f1b_s[:, t:t + 1],
                                            0.0, OP.add, OP.max)
            pso = ppH.tile([128, 256], F32, tag="psH")
            for t in range(8):
                nc.tensor.matmul(pso[:], lhsT=hT[:, t, :], rhs=f2w_s[:, t, :],
                                 start=(t == 0), stop=(t == 7))
            y2 = sp.tile([128, 256], F32, tag="y2")
            nc.vector.tensor_add(y2[:], pso[:], x1[:])
            nc.vector.tensor_add(y2[:], y2[:], f2b_s[:])
            x2 = layer_norm(y2, l2g_s, l2b_s, "ln2")
            nc.scalar.dma_start(out=out_d[:], in_=x2[:])

    _split_multiwaits(nc)
    return nc


# ---------------------------------------------------------------------------
def prep_inputs(inp):
    f32 = np.float32
    f16 = np.float16
    q_asn = np.asarray(inp["q_assignments"], f32)
    k_asn = np.asarray(inp["k_assignments"], f32)
    query = np.asarray(inp["query"], f32)
    key = np.asarray(inp["key_emb"], f32)

    def xfeat(coords, iso):
        oh = np.zeros((N, NISO), f32)
        oh[np.arange(N), np.asarray(iso) - 1] = 1.0
        x = np.concatenate([oh, np.ones((N, 1), f32), np.asarray(coords, f32),
                            np.zeros((N, 1), f32)], axis=1)
        return np.ascontiguousarray(
            x.reshape(NCH, 128, XC).transpose(1, 0, 2)).astype(NP8)

    qx8 = xfeat(inp["q_coords"], inp["q_iso"])
    kx8 = xfeat(inp["k_coords"], inp["k_iso"])

    g1 = np.asarray(inp["g1_w"], f32)
    g1b = np.asarray(inp["g1_b"], f32)
    g2 = np.asarray(inp["g2_w"], f32)[:, 0]
    g2b = float(np.asarray(inp["g2_b"], f32)[0])
    h1 = np.asarray(inp["h1_w"], f32)
    h1b = np.asarray(inp["h1_b"], f32)
    h2 = np.asarray(inp["h2_w"], f32)[:, 0]
    h2b = float(np.asarray(inp["h2_b"], f32)[0])

    ag, sg = np.abs(g2), np.sign(g2).astype(f32)
    ah, sh = np.abs(h2), np.sign(h2).astype(f32)

    g1ab = np.concatenate([g1[:12] * ag[None, :], (g1b * ag)[None, :]], axis=0)
    g1b4 = np.tile(g1[12:] * ag[None, :], (1, 4))

    hpatV = np.zeros((80, HG, 128), f32)
    for gp in range(HG):
        for ii in range(4):
            hpatV[5 * gp + ii, gp, 32 * ii:32 * (ii + 1)] = h1[0] * ah
            hpatV[5 * gp + 4, gp, 32 * ii:32 * (ii + 1)] = h1[2] * ah
    wq = (h1[1] * ah)[None, :].astype(f32)
    b1r = (h1b * ah)[None, :].astype(f32)

    # H contract: DoubleRow pattern per global pair pr (groups 2pr, 2pr+1),
    # writing gin/hin columns 4g..4g+3.
    cchDR = np.zeros((128, HG, 2, 128), f32)
    for pr in range(HG):
        for t in range(2):
            gg = 2 * pr + t
            for ii in range(4):
                for c in range(32):
                    cchDR[32 * ii + c, pr, t, 4 * gg + ii] = sh[c]
    cchDR = cchDR.astype(NP8)

    ccg16 = np.zeros((128, 2 * HG, 128), f32)
    for g in range(2 * HG):
        for ii in range(4):
            for c in range(32):
                ccg16[32 * ii + c, g, 4 * g + ii] = sg[c]
    ccg16 = ccg16.astype(f16)

    permV = np.zeros((HW, 80), f32)
    for i in range(HW):
        permV[i, 5 * (i // 4) + (i % 4)] = 1.0
    insV = np.zeros((1, 80), f32)
    insV[0, 4::5] = 1.0

    sens = np.asarray(inp["sensor_coords"], f32)[None, :]
    sensr = np.tile(sens, (HW, 1))

    f1wt = np.ascontiguousarray(
        np.asarray(inp["ffn1_w"], f32).reshape(2, 128, 8, 128).transpose(1, 0, 2, 3)).astype(f16)
    f1b = np.ascontiguousarray(np.asarray(inp["ffn1_b"], f32).reshape(8, 128).T)
    f2wt = np.ascontiguousarray(
        np.asarray(inp["ffn2_w"], f32).reshape(8, 128, 256).transpose(1, 0, 2)).astype(f16)
    f2br = np.tile(np.asarray(inp["ffn2_b"], f32)[None, :], (128, 1))
    l1g = np.tile(np.asarray(inp["ln1_g"], f32)[None, :], (128, 1))
    l1b = np.tile(np.asarray(inp["ln1_b"], f32)[None, :], (128, 1))
    l2g = np.tile(np.asarray(inp["ln2_g"], f32)[None, :], (128, 1))
    l2b = np.tile(np.asarray(inp["ln2_b"], f32)[None, :], (128, 1))

    keyT3 = np.ascontiguousarray(key.T.reshape(2, 128, 1024).transpose(1, 0, 2)).astype(f16)
    key3 = np.ascontiguousarray(key.reshape(8, 128, 256).transpose(1, 0, 2)).astype(f16)

    shared = {
        "qx8": qx8, "kx8": kx8,
        "keyT3": keyT3, "key3": key3,
        "g1ab": g1ab, "g1b4": g1b4.astype(f16),
        "hpatV": hpatV.astype(f16), "wq_h": wq, "b1_h": b1r,
        "g2b": np.full((128, 1), g2b, f32), "h2b": np.full((128, 1), h2b, f32),
        "cchDR": cchDR, "ccg16": ccg16,
        "permV": permV.astype(f16), "insV": insV.astype(f16),
        "ones16": np.ones((1, 128), f16), "ones32": np.ones((1, 64), f32),
        "i64f": np.eye(64, dtype=f32), "i128f": np.eye(128, dtype=f32),
        "i128h": np.eye(128, dtype=f16), "sensr128": np.tile(sens, (128, 1)),
        "sens": sens, "sensr": sensr, "onec": np.ones((64, 1), f32),
        "f1w": f1wt, "f1b": f1b, "f2w": f2wt, "f2br": f2br,
        "l1g": l1g, "l1b": l1b, "l2g": l2g, "l2b": l2b,
        "epsc": np.full((128, 1), 1e-6, f32),
    }

    in_maps = []
    for m in range(NCORES):
        sl = slice(m * KSL, (m + 1) * KSL)
        qa = (q_asn[:, sl] * ASCALE).astype(NP8)
        ka = (k_asn[:, sl] * ASCALE).astype(NP8)
        qa8 = np.ascontiguousarray(
            qa.reshape(NCH, 128, NH, HW).transpose(1, 2, 0, 3))
        ka8 = np.ascontiguousarray(
            ka.reshape(NCH, 128, 128).transpose(1, 0, 2))
        qT3 = np.ascontiguousarray(
            query[sl].T.reshape(2, 128, 128).transpose(1, 0, 2)).astype(f16)
        im = dict(shared)
        im.update({
            "qa8": qa8, "ka8": ka8, "qT3": qT3,
            "q_sbh": np.ascontiguousarray(query[sl]),
        })
        in_maps.append(im)
    return in_maps


def kernel(**inputs) -> np.ndarray:
    if "nc" not in _cache:
        _cache["nc"] = build_program()
    nc = _cache["nc"]
    in_maps = prep_inputs(inputs)
    res = run_bass_kernel_spmd(nc, in_maps, list(range(NCORES)))
    return np.concatenate([res.results[m]["out"] for m in range(NCORES)], axis=0)
